# revision 8
# baseline (speedup 1.0000x reference)
"""Trainium2 Bass kernel for a 2-layer mean-aggregation GraphSAGE GNN.

Strategy (8 NeuronCores, SPMD single program):
  - Destination nodes sharded contiguously across cores (6250/core), with
    a host-side node relabeling that (a) balances total in-degree across
    cores and (b) packs each 64-dst PSUM window to <= 1024 incoming edges
    (greedy + swap repair), so the max-over-cores window budgets hit the
    128-slot quantization exactly (L1 stream = 100096 slots vs 112512
    unbalanced).
  - Features / hidden activations stored as FP16 PAIR tables ([n/2, 128]
    rows = two consecutive node rows = 256B, the dma_gather granularity),
    so gather indices are pair ids < 32768 (int16) and no A/B table split
    is needed.  Parity (which pair half) is handled per chunk: where the
    even/odd split costs no padding the slots are parity-sorted and each
    chunk takes ONE fp16 matmul with the matching lhsT half; otherwise
    parity folds into the dstrel (+64) compared against a width-128 iota
    with two matmuls per chunk.
  - One-hots are built per chunk with tensor_scalar(is_equal) in fp16 —
    all-SBUF 2-byte operands hit the DVE 4x perf mode (~93ns/chunk).
    Mean division is applied per 512-col slab after the psum closes.
  - The gather/onehot/matmul stream is software-pipelined (front-end 2
    calls ahead), so close-op stalls at the DVE queue head never starve
    the PE; input loads are staged in three waves so the DMA-engine FIFO
    head stays clear for the gathers feeding the first AllGather.
  - PSUM: two windows per [128, 512] bank tile ((w%2)*64 partition half,
    ((w//2)%8)*64 column), rolling groups of 16 windows; each stream
    closes a group with ONE strided ACT copy (or DVE add for later
    layer-2 pieces) per partition half — accumulation groups stay
    contiguous per stream (required: split start/stop groups corrupt).
  - h exchange: 3 AllGathers over fp16 pair-row pieces of the local node
    range (boundaries chosen per input from candidates, slab-aligned),
    emitted a few calls after their h piece completes so their sem waits
    never head-block the Pool queue; layer-2 slot streams are split by
    src piece so gathers fire as each AllGather lands.
  - Projections via float32r matmuls over 512-col slabs (1 cycle/row).
  - Final [32, 6250] per-core output is transposed/concatenated and
    un-permuted on host.
"""

import os
import sys

import numpy as np

for _p in ("/opt/trn_rl_repo", "/root/.axon_site/_ro/trn_rl_repo"):
    if os.path.isdir(_p) and _p not in sys.path:
        sys.path.append(_p)

# ---- problem constants (hardcoded per harness contract) ----
N_NODES = 50000
N_EDGES = 800000
IN_F = 64
HID = 64
OUT_C = 32
M_CORES = 8
NPC = N_NODES // M_CORES     # 6250
WIN = 64
NW = -(-NPC // WIN)          # 98
GB = 2048                    # slots per dma_gather call
SLAB = 512                   # projection slab (psum bank cols)
# piece boundaries must be SLAB multiples (h DMAs never straddle pieces)
PIECE_CANDS = [(2048, 4096), (1536, 4096), (2048, 4608), (2560, 4608),
               (2048, 3584), (3072,), (3584,), (2560,)]
AG_AFTER_P1_CALLS = 8        # issue last AllGather after this many L2p1 calls
AG_DELAY = int(os.environ.get('AG_DELAY', 3))   # calls between h-piece completion and its AG
NS_PER_SLOT = 1.4225e-3      # µs, gather DMA cost per slot (cost model)


def _round_up(x, k):
    return (x + k - 1) // k * k


def _balance_permutation(deg):
    """Relabel nodes so per-core and per-window in-degree sums are flat,
    minimizing the round-128 padding of the max-over-cores window budgets.
    Returns perm (new position -> original node id)."""
    import heapq

    order = np.argsort(-deg, kind="stable")
    # 1) balance total in-degree across cores (capacity NPC each)
    heap = [(0, c) for c in range(M_CORES)]
    heapq.heapify(heap)
    cap = [NPC] * M_CORES
    core_nodes = [[] for _ in range(M_CORES)]
    for v in order:
        picked = []
        while True:
            s, c = heapq.heappop(heap)
            if cap[c] > 0:
                break
            picked.append((s, c))
        core_nodes[c].append(v)
        cap[c] -= 1
        heapq.heappush(heap, (s + int(deg[v]), c))
        for it in picked:
            heapq.heappush(heap, it)

    # 2) within each core, pack windows toward exact 1024-edge targets
    perm = np.empty(N_NODES, np.int64)
    lastcap = NPC - (NW - 1) * WIN
    for c in range(M_CORES):
        nodes = core_nodes[c]                     # desc by degree
        tot = int(deg[nodes].sum())
        t_last = max(tot - (NW - 1) * 8 * 128, 8 * int(deg[nodes[0]]))
        # heap of (-slack, w); assign heaviest node to max-slack window
        caps = [WIN] * (NW - 1) + [lastcap]
        tgts = [8 * 128] * (NW - 1) + [t_last]
        sums = [0] * NW
        cnts = [0] * NW
        heap2 = [(-tgts[w], w) for w in range(NW)]
        heapq.heapify(heap2)
        wassign = [[] for _ in range(NW)]
        for v in nodes:
            picked = []
            while True:
                negslack, w = heapq.heappop(heap2)
                if cnts[w] < caps[w]:
                    break
                picked.append((negslack, w))
            wassign[w].append(v)
            cnts[w] += 1
            sums[w] += int(deg[v])
            heapq.heappush(heap2, (sums[w] - tgts[w], w))
            for it in picked:
                heapq.heappush(heap2, it)
        # 3) repair pass: swap nodes between windows so no full window
        # exceeds the 1024-edge (8-chunk) budget bin; the short last
        # window may absorb up to its 768 bin.
        BIN = 8 * 128
        from collections import defaultdict
        wdeg = [defaultdict(list) for _ in range(NW)]   # deg -> [nodes]
        for w in range(NW):
            for v in wassign[w]:
                wdeg[w][int(deg[v])].append(v)

        def room(u):
            cap_u = BIN if u < NW - 1 else 768
            return cap_u - sums[u]

        for w in range(NW - 1):
            guard = 0
            while sums[w] > BIN and guard < 200:
                guard += 1
                need = sums[w] - BIN
                done = False
                for da in sorted(wdeg[w], reverse=True):
                    if not wdeg[w][da]:
                        continue
                    for k in range(min(need, da - 1), 0, -1):
                        db = da - k
                        for u in range(NW):
                            if (u == w or room(u) < k
                                    or not wdeg[u].get(db)):
                                continue
                            a = wdeg[w][da].pop()
                            b = wdeg[u][db].pop()
                            wdeg[w][db].append(b)
                            wdeg[u][da].append(a)
                            ia = wassign[w].index(a)
                            ib = wassign[u].index(b)
                            wassign[w][ia] = b
                            wassign[u][ib] = a
                            sums[w] -= k
                            sums[u] += k
                            done = True
                            break
                        if done:
                            break
                    if done:
                        break
                if not done:
                    break

        # 4) parity balance: within each window, order nodes so the
        # even-position half and odd-position half carry (<= bud/2) each
        # -> layer-1 chunks are parity-pure with zero extra padding
        for w in range(NW):
            nodes = sorted(wassign[w], key=lambda v: -deg[v])
            hcap = _round_up(max(sums[w], 1), 128) // 2
            bins = [[], []]
            bsum = [0, 0]
            bcap = [(len(nodes) + 1) // 2, len(nodes) // 2]
            for v in nodes:
                t = 0 if (bsum[0] <= bsum[1] and len(bins[0]) < bcap[0]) \
                    else (1 if len(bins[1]) < bcap[1] else 0)
                bins[t].append(v)
                bsum[t] += int(deg[v])
            for _ in range(64):
                hi = 0 if bsum[0] >= bsum[1] else 1
                excess = bsum[hi] - hcap
                if excess <= 0:
                    break
                done = False
                for a in sorted(bins[hi], key=lambda v: -deg[v]):
                    for b in sorted(bins[1 - hi], key=lambda v: deg[v]):
                        k = int(deg[a]) - int(deg[b])
                        if 1 <= k <= excess and bsum[1 - hi] + k <= hcap:
                            bins[hi].remove(a)
                            bins[1 - hi].remove(b)
                            bins[hi].append(b)
                            bins[1 - hi].append(a)
                            bsum[hi] -= k
                            bsum[1 - hi] += k
                            done = True
                            break
                    if done:
                        break
                if not done:
                    break
            merged = []
            for i in range(len(nodes)):
                merged.append(bins[i % 2][i // 2])
            wassign[w] = merged

        pos = c * NPC
        for w in range(NW):
            for v in wassign[w]:
                perm[pos] = v
                pos += 1
        assert pos == (c + 1) * NPC
    return perm


def _stream_layout(core, w, par, rel, gidx):
    """Parity-sorted padded slot streams for one gather stream.

    Within each window segment: even-parity slots first, padded to the
    shared boundary E_w = max-over-cores even count; odd slots after.
    Chunks below/above the boundary are parity-pure and take ONE matmul
    with the matching lhsT half; the (single) straddling chunk is mixed
    and uses the parity-folded dstrel + two matmuls.

    Returns (budgets_chunks[NW], kinds per chunk 'E'/'O'/'M',
             idx [M, S], drl [M, S], S).
    """
    ce = np.zeros((M_CORES, NW), np.int64)
    co = np.zeros((M_CORES, NW), np.int64)
    np.add.at(ce, (core[par == 0], w[par == 0]), 1)
    np.add.at(co, (core[par == 1], w[par == 1]), 1)
    E = ce.max(axis=0)
    O = co.max(axis=0)
    J = (ce + co).max(axis=0)
    bud = np.maximum(_round_up(J, 128), 128)
    # sorted mode only where the parity split costs no extra padding;
    # otherwise fold parity into dstrel (all chunks mixed)
    smode = _round_up(E + O, 128) <= bud
    # even/odd boundary: chunk-aligned when the odds still fit
    B = np.where(_round_up(E, 128) + O <= bud, _round_up(E, 128), E)
    B = np.where(smode, B, 0)
    seg_off = np.concatenate([[0], np.cumsum(bud)])
    S = int(seg_off[-1])

    # chunk kinds (shared across cores)
    kinds = []
    for wi in range(NW):
        nch = int(bud[wi]) // 128
        for j in range(nch):
            lo, hi = j * 128, (j + 1) * 128
            if not smode[wi]:
                kinds.append("M")
            elif hi <= B[wi]:
                kinds.append("E")
            elif lo >= B[wi]:
                kinds.append("O")
            else:
                kinds.append("M")

    # slot positions: sorted windows rank evens from seg start, odds
    # from B; folded windows rank jointly
    pcls = np.where(smode[w], par, 0)
    key = (core * NW + w) * 2 + pcls
    order = np.argsort(key, kind="stable")
    ks = key[order]
    grp_start = np.searchsorted(ks, np.arange(M_CORES * NW * 2), side="left")
    ranks = np.arange(len(ks)) - grp_start[ks]
    wo = w[order]
    po = par[order]
    pos = seg_off[wo] + np.where(smode[wo] & (po == 1), B[wo], 0) + ranks

    # dstrel: fold +64*parity for folded windows and mixed-chunk odds
    ch_in_seg = (pos - seg_off[wo]) // 128
    mixed = (ch_in_seg * 128 < B[wo]) & ((ch_in_seg + 1) * 128 > B[wo])
    fold = (~smode[wo]) | (mixed & (po == 1))
    drl_o = np.where(fold & (po == 1), rel[order] + WIN,
                     rel[order]).astype(np.float32)

    idx_buf = np.zeros((M_CORES, S), np.int64)
    drl_buf = np.full((M_CORES, S), -1.0, np.float32)
    idx_buf[core[order], pos] = gidx[order]
    drl_buf[core[order], pos] = drl_o
    return [int(b) // 128 for b in bud], kinds, idx_buf, drl_buf, S


def _wrap_idx(streams):
    """[M, S] int -> per-core [128, S/16] int16 gather-index layout."""
    res = []
    for c in range(M_CORES):
        a = streams[c].astype(np.int16).reshape(-1, 16).T
        res.append(np.ascontiguousarray(np.tile(a, (8, 1))))
    return res


def _prep(src, dst):
    deg = np.bincount(dst, minlength=N_NODES).astype(np.int64)
    invdeg = (1.0 / np.maximum(deg, 1.0)).astype(np.float32)

    core = dst // NPC
    dloc = dst % NPC
    w_e = dloc // WIN
    par = (src & 1).astype(np.int64)
    drel = (dloc % WIN + WIN * par).astype(np.float32)
    sloc = src % NPC
    scor = src // NPC

    # ---- L1: single stream, gather from global x pair table ----
    rel = (dloc % WIN).astype(np.float32)
    bud1, kinds1, idx1_buf, drl1_buf, S1 = _stream_layout(
        core, w_e, par, rel, src >> 1)

    # ---- choose L2 piece boundaries (min estimated critical chain) ----
    l1_dma = S1 * NS_PER_SLOT            # µs
    best = None
    for bounds in PIECE_CANDS:
        offs = [0] + list(bounds) + [NPC]
        piece_slots = []
        tot = 0
        for p in range(len(offs) - 1):
            sel = (sloc >= offs[p]) & (sloc < offs[p + 1])
            c = np.zeros((M_CORES, NW), np.int64)
            np.add.at(c, (core[sel], w_e[sel]), 1)
            b = _round_up(c.max(axis=0), 128)
            if p == 0:
                b = np.maximum(b, 128)
            piece_slots.append(int(b.sum()))
            tot += int(b.sum())
        # chain: AGs serialize on the collective resource; last piece's
        # gathers wait for its AG
        t = 0.0
        for p in range(len(offs) - 1):
            ready = offs[p + 1] / NPC * l1_dma + 18.0
            nodes = offs[p + 1] - offs[p]
            ag = 15.0 + nodes * 8 * 128 / 40e3
            t = max(t, ready) + ag
        span = max(t + piece_slots[-1] * NS_PER_SLOT + 12.0,
                   (S1 + tot) * NS_PER_SLOT + 25.0)
        if best is None or span < best[0]:
            best = (span, offs)
    if os.environ.get("FORCE_OFFS"):
        best = (0, [0] + [int(x) for x in
                          os.environ["FORCE_OFFS"].split(",")] + [NPC])
    offs = best[1]
    npieces = len(offs) - 1
    assert all(o % SLAB == 0 for o in offs[1:-1])

    # ---- L2: one stream per piece, gather from h pair-piece tables ----
    bud2, kinds2, S2 = [], [], []
    idx2_bufs, drl2_bufs = [], []
    for p in range(npieces):
        np_p = (offs[p + 1] - offs[p]) // 2          # pairs/core in piece
        sel = (sloc >= offs[p]) & (sloc < offs[p + 1])
        gidx = scor[sel] * np_p + (sloc[sel] - offs[p]) // 2
        assert gidx.max() < 8 * np_p <= 32768
        b, kn, ib, db, S = _stream_layout(
            core[sel], w_e[sel], par[sel], rel[sel], gidx)
        bud2.append(b)
        kinds2.append(kn)
        idx2_bufs.append(ib)
        drl2_bufs.append(db)
        S2.append(S)

    st = dict(
        offs=offs, npieces=npieces,
        bud1=bud1, bud2=bud2,
        kinds1=kinds1, kinds2=kinds2,
        S1=S1, S2=S2,
        nch1=S1 // 128, nch2=[s // 128 for s in S2],
    )

    # per-core drt: [128, nch1 + sum(nch2)] f32 (L1 cols then L2 pieces)
    drt = []
    for c in range(M_CORES):
        cols = [drl1_buf[c].reshape(-1, 128).T]
        for p in range(npieces):
            cols.append(drl2_bufs[p][c].reshape(-1, 128).T)
        drt.append(np.ascontiguousarray(np.concatenate(cols, axis=1)))

    idx1 = _wrap_idx(idx1_buf)
    idx2 = []
    for c in range(M_CORES):
        blocks = [idx2_bufs[p][c] for p in range(npieces)]
        flat = np.concatenate(blocks)
        a = flat.astype(np.int16).reshape(-1, 16).T
        idx2.append(np.ascontiguousarray(np.tile(a, (8, 1))))

    pc = dict(drt=drt, idx1=idx1, idx2=idx2, invdeg=invdeg)
    return st, pc


def _build_bass(st, debug=False):
    import concourse.bass as bass
    import concourse.mybir as mybir
    import concourse.tile as tile
    from concourse.ap import AP
    from concourse import bacc, library_config

    f32 = mybir.dt.float32
    f32r = mybir.dt.float32r
    fp16 = mybir.dt.float16
    i16 = mybir.dt.int16
    AF = mybir.ActivationFunctionType
    OP = mybir.AluOpType

    offs = st["offs"]
    npieces = st["npieces"]
    nch1 = st["nch1"]
    nch2 = st["nch2"]
    NCH = nch1 + sum(nch2)
    np_p = [(offs[p + 1] - offs[p]) // 2 for p in range(npieces)]

    nc = bacc.Bacc(None, target_bir_lowering=False)

    xpair_d = nc.dram_tensor("xpair", [N_NODES // 2, 128], fp16,
                             kind="ExternalInput")
    xT_d = nc.dram_tensor("xT", [IN_F, NPC], f32r, kind="ExternalInput")
    w1c_d = nc.dram_tensor("w1c", [2 * IN_F, HID], f32r, kind="ExternalInput")
    w2c_d = nc.dram_tensor("w2c", [2 * HID, OUT_C], f32r, kind="ExternalInput")
    b1_d = nc.dram_tensor("b1c", [HID, 1], f32, kind="ExternalInput")
    b2_d = nc.dram_tensor("b2c", [OUT_C, 1], f32, kind="ExternalInput")
    iota_d = nc.dram_tensor("iota", [128, 128], fp16, kind="ExternalInput")
    ident_d = nc.dram_tensor("ident", [IN_F, IN_F], f32, kind="ExternalInput")
    invd_d = nc.dram_tensor("invd", [64, NPC], f32, kind="ExternalInput")
    drel_d = nc.dram_tensor("dstrel", [128, NCH], f32, kind="ExternalInput")
    idx1_d = nc.dram_tensor("idx1", [128, st["S1"] // 16], i16,
                            kind="ExternalInput")
    idx2_d = nc.dram_tensor("idx2", [128, sum(st["S2"]) // 16], i16,
                            kind="ExternalInput")
    out_d = nc.dram_tensor("out", [OUT_C, NPC], f32, kind="ExternalOutput")
    scratch_d = nc.dram_tensor("scratch", [1, 64], fp16)
    if debug:
        dbg_z1_d = nc.dram_tensor("dbg_z1", [128, NPC], f32,
                                  kind="ExternalOutput")
        dbg_z2_d = nc.dram_tensor("dbg_z2", [128, NPC], f32,
                                  kind="ExternalOutput")

    h_shard = [nc.dram_tensor(f"h_shard_{p}", [2 * np_p[p], HID], fp16)
               for p in range(npieces)]
    h_table = [nc.dram_tensor(f"h_table_{p}", [2 * 8 * np_p[p], HID], fp16,
                              addr_space="Shared")
               for p in range(npieces)]

    with tile.TileContext(nc) as tc:
        nc.gpsimd.load_library(library_config.mlp)
        with (
            tc.tile_pool(name="const", bufs=1) as cpool,
            tc.tile_pool(name="gath", bufs=int(os.environ.get("GBUFS", 4))) as gpool,
            tc.tile_pool(name="oh", bufs=int(os.environ.get("GBUFS", 4))) as ohpool,
            tc.tile_pool(name="hsb", bufs=3) as hpool,
            tc.tile_pool(name="osl", bufs=2) as opool,
            tc.tile_pool(name="agg", bufs=6, space="PSUM") as apool,
            tc.tile_pool(name="msc", bufs=1, space="PSUM") as mpool,
            tc.tile_pool(name="prj", bufs=1, space="PSUM") as jpool,
        ):
            # ---- persistent SBUF ----
            z1 = cpool.tile([2 * IN_F, NPC], f32r, tag="z1")
            z2 = cpool.tile([2 * HID, NPC], f32r, tag="z2")
            w1t = cpool.tile([2 * IN_F, HID], f32r, tag="w1t")
            w2t = cpool.tile([2 * HID, OUT_C], f32r, tag="w2t")
            b1t = cpool.tile([HID, 1], f32, tag="b1t")
            b2t = cpool.tile([OUT_C, 1], f32, tag="b2t")
            iot = cpool.tile([128, 128], fp16, tag="iot")
            idt = cpool.tile([IN_F, IN_F], f32, tag="idt")
            ivt = cpool.tile([128, NPC], f32, tag="ivt")
            drt = cpool.tile([128, NCH], f32, tag="drt")
            ix1 = cpool.tile([128, st["S1"] // 16], i16, tag="ix1")
            ix2 = cpool.tile([128, sum(st["S2"]) // 16], i16, tag="ix2")

            # load order matters: the DMA engines are FIFO, so the first
            # gather call queues behind whatever consts precede it.  Load
            # only the L1-stream-critical prefix first (idx/dstrel split
            # so subtile deps release the first gather early); defer the
            # L2-only loads behind the first h write (SP queue blocks on
            # it, staggering them off the head of the DMA FIFO).
            # staged input loads: the DMA engines are FIFO, so anything
            # loaded before the gathers a piece-1 AllGather depends on
            # delays the whole collective chain.  stage0 = minimal stream
            # prefix; stage2 (after call 1, held back by a blocker DMA
            # reading call-1's onehot) = what layer-1 slabs 0..7 need;
            # stage3 (after piece-1's h is written) = everything else.
            IX1H = min(2560 // 16 * 16, st["S1"] // 16)
            DRTH = min(320, NCH)
            XTH = min(4096, NPC)
            nc.sync.dma_start(ix1[:, 0:IX1H], idx1_d[:, 0:IX1H])
            nc.sync.dma_start(iot[:], iota_d[:])
            nc.sync.dma_start(drt[:, 0:DRTH], drel_d[:, 0:DRTH])

            last_oh = [None]
            stage2 = [False]

            def load_stage2():
                nc.sync.dma_start(scratch_d[0:1, :],
                                  last_oh[0][0:1, 0, 0:64])
                nc.sync.dma_start(w1t[:], w1c_d[:])
                nc.sync.dma_start(b1t[:], b1_d[:])
                nc.sync.dma_start(z1[0:IN_F, 0:XTH], xT_d[:, 0:XTH])
                nc.sync.dma_start(ivt[64:128, 0:XTH], invd_d[:, 0:XTH])
                nc.sync.dma_start(idt[:], ident_d[:])
                stage2[0] = True

            deferred = [False]

            def load_deferred():
                nc.sync.dma_start(ix1[:, IX1H:], idx1_d[:, IX1H:])
                nc.sync.dma_start(drt[:, DRTH:], drel_d[:, DRTH:])
                nc.sync.dma_start(z1[0:IN_F, XTH:], xT_d[:, XTH:])
                nc.sync.dma_start(ivt[64:128, XTH:], invd_d[:, XTH:])
                nc.sync.dma_start(ix2[:], idx2_d[:])
                nc.sync.dma_start(w2t[:], w2c_d[:])
                nc.sync.dma_start(b2t[:], b2_d[:])
                deferred[0] = True

            # misc psum bank: [:, 0:64] = windows 96/97, [:, 64+64j] = L1
            # transpose slots
            misc = mpool.tile([128, SLAB], f32, tag="misc")

            # ---------------- shared machinery ----------------
            def win_slice(wtiles, w, wn):
                t = w // 2
                if t == 48:
                    tl = misc
                    col = 0
                else:
                    tl = wtiles[t // 8]
                    col = (t % 8) * 64
                ph = (w % 2) * 64
                return tl[ph: ph + 64, col: col + wn]

            def emit_stream(chunks_per_win, kinds, kbase, ixt, ix_off,
                            table_ap, wtiles, alloc_group, on_call_end):
                """Issue gather/onehot/matmul stream, software-pipelined:
                gathers + onehots run PIPE calls ahead of the matmuls, so
                a close-op stall at the DVE queue head never starves the
                PE.  Each window's psum accumulation group is contiguous
                WITHIN this stream.  Parity-pure chunks ('E'/'O') take one
                matmul with the matching lhsT half; mixed chunks use the
                folded dstrel + two."""
                chlist = []
                for w in range(NW):
                    for j in range(chunks_per_win[w]):
                        chlist.append((w, j))
                calls = []
                k = 0
                while k < len(chlist):
                    nb = min(GB // 128, len(chlist) - k)
                    calls.append((k, nb))
                    k += nb
                PIPE = int(os.environ.get('PIPE', 2))
                tiles = {}

                def front(ci):
                    k, nb = calls[ci]
                    b0 = k * 128
                    g = gpool.tile([128, GB // 128, 128], fp16, tag="g",
                                   name="g")
                    nc.gpsimd.dma_gather(
                        out_ap=g[:, 0:nb, :],
                        in_ap=table_ap,
                        idxs_ap=ixt[:, ix_off + b0 // 16:
                                    ix_off + b0 // 16 + nb * 8],
                        num_idxs=nb * 128,
                        num_idxs_reg=nb * 128,
                        elem_size=128,
                        single_packet=False,
                    )
                    oh = ohpool.tile([128, GB // 128, 128], fp16, tag="oh",
                                     name="oh")
                    last_oh[0] = oh
                    for col in range(nb):
                        kind = kinds[k + col]
                        ohw = 128 if kind == "M" else 64
                        # pure onehot (iota == dstrel), fp16, 4x DVE mode
                        nc.vector.tensor_scalar(
                            out=oh[:, col, 0:ohw],
                            in0=iot[:, 0:ohw],
                            scalar1=drt[:, kbase + k + col:
                                        kbase + k + col + 1],
                            scalar2=None,
                            op0=OP.is_equal,
                        )
                    tiles[ci] = (g, oh)

                def back(ci):
                    k, nb = calls[ci]
                    g, oh = tiles.pop(ci)
                    for col in range(nb):
                        w, j = chlist[k + col]
                        kind = kinds[k + col]
                        wn = min(WIN, NPC - w * WIN)
                        if (w // 2) != 48 and (w // 16) not in wtiles:
                            wtiles[w // 16] = alloc_group()
                        ps = win_slice(wtiles, w, wn)
                        first = (j == 0)
                        last = (j == chunks_per_win[w] - 1)
                        if kind == "M":
                            nc.tensor.matmul(
                                ps, g[:, col, 0:64], oh[:, col, 0:wn],
                                start=first, stop=False)
                            nc.tensor.matmul(
                                ps, g[:, col, 64:128],
                                oh[:, col, 64:64 + wn],
                                start=False, stop=last)
                        else:
                            half = slice(0, 64) if kind == "E" \
                                else slice(64, 128)
                            nc.tensor.matmul(
                                ps, g[:, col, half], oh[:, col, 0:wn],
                                start=first, stop=last)
                    on_call_end(ci + 1, chlist[k + nb - 1][0])

                for ci in range(len(calls)):
                    front(ci)
                    if ci >= PIPE:
                        back(ci - PIPE)
                for ci in range(max(0, len(calls) - PIPE), len(calls)):
                    back(ci)

            def close_groups(layer, z, wtiles, upto_g, state, add=False,
                             run_slabs=True):
                """Close whole 16-window psum groups <= upto_g: one strided
                ACT copy (or DVE add) per (bank tile, partition half)
                moves 8 windows at once; then run slab completions."""
                ng = 7  # groups 0..5 full tiles, group 6 = windows 96/97
                while state["g"] <= min(upto_g, ng - 1):
                    g = state["g"]
                    if g < 6:
                        tl = wtiles[g]
                        for ph in (0, 64):
                            # even (ph=0) / odd (ph=64) windows of group
                            zb = z[HID:, (16 * g + ph // 64) * WIN:
                                   (16 * g + ph // 64) * WIN + WIN]
                            zsl = AP(zb.tensor, zb.offset,
                                     [zb.ap[0], [128, 8], [1, WIN]])
                            pb = tl[ph: ph + 64, 0:SLAB]
                            psl = AP(pb.tensor, pb.offset,
                                     [pb.ap[0], [WIN, 8], [1, WIN]])
                            if not add:
                                nc.scalar.copy(zsl, psl)
                            else:
                                nc.vector.scalar_tensor_tensor(
                                    out=zsl, in0=psl, scalar=1.0, in1=zsl,
                                    op0=OP.mult, op1=OP.add)
                    else:
                        for w in (96, 97):
                            wn = min(WIN, NPC - w * WIN)
                            zsl = z[HID:, w * WIN: w * WIN + wn]
                            ps = win_slice(wtiles, w, wn)
                            if not add:
                                nc.scalar.copy(zsl, ps)
                            else:
                                nc.vector.scalar_tensor_tensor(
                                    out=zsl, in0=ps, scalar=1.0, in1=zsl,
                                    op0=OP.mult, op1=OP.add)
                    state["g"] += 1
                    if run_slabs:
                        for s in (2 * g, 2 * g + 1):
                            if s * SLAB < NPC:
                                finish_slab(layer, z, s)

            def finish_slab(layer, z, s):
                a, b = s * SLAB, min((s + 1) * SLAB, NPC)
                cols = b - a
                # mean division (invdeg folded out of the onehots)
                nc.vector.scalar_tensor_tensor(
                    out=z[HID:, a:b], in0=z[HID:, a:b], scalar=1.0,
                    in1=ivt[64:128, a:b].bitcast(f32r),
                    op0=OP.mult, op1=OP.mult)
                if layer == 1:
                    p1 = jpool.tile([HID, SLAB], f32, tag="prj",
                                    name="prj")
                    nc.tensor.matmul(p1[:, :cols], w1t[:], z[:, a:b],
                                     start=True, stop=True)
                    nc.scalar.activation(z2[0:HID, a:b], p1[:, :cols],
                                         AF.Relu, bias=b1t[:, 0:1])
                    for j4 in range(-(-cols // 128)):
                        ca = a + j4 * 128
                        cb = min(ca + 128, b)
                        cc = cb - ca
                        tp = misc[:, 64 + (j4 % 4) * 64:
                                  128 + (j4 % 4) * 64]
                        nc.tensor.transpose(tp[:cc, :],
                                            z2[0:HID, ca:cb].bitcast(f32),
                                            idt[:])
                        hs = hpool.tile([128, HID], fp16, tag="hs",
                                        name="hs")
                        nc.scalar.copy(hs[:cc, :], tp[:cc, :])
                        # piece containing this chunk
                        p = next(i for i in range(npieces)
                                 if offs[i] <= ca < offs[i + 1])
                        nc.sync.dma_start(
                            h_shard[p][ca - offs[p]: cb - offs[p], :],
                            hs[:cc, :])
                    if not deferred[0] and b >= offs[1]:
                        load_deferred()
                    # queue AllGathers for completed h pieces (all but the
                    # last, which is deferred into the L2 piece-0 stream)
                    for p in range(npieces - 1):
                        if not ag_emitted[p] and b >= offs[p + 1]:
                            ag_pending.append(
                                (p, ag_ready[-1] if ag_ready else 0))
                            ag_emitted[p] = True
                else:
                    p2 = jpool.tile([HID, SLAB], f32, tag="prj",
                                    name="prj")[0:OUT_C, :]
                    nc.tensor.matmul(p2[:, :cols], w2t[:], z[:, a:b],
                                     start=True, stop=True)
                    osl = opool.tile([OUT_C, SLAB], f32, tag="osl",
                                     name="osl")
                    nc.scalar.activation(osl[:, :cols], p2[:, :cols],
                                         AF.Identity, bias=b2t[:, 0:1])
                    nc.sync.dma_start(out_d[:, a:b], osl[:, :cols])

            def emit_ag(p):
                nc.gpsimd.collective_compute(
                    "AllGather",
                    mybir.AluOpType.bypass,
                    replica_groups=[list(range(M_CORES))],
                    ins=[h_shard[p][:]],
                    outs=[h_table[p][:]],
                )

            # ================= layer 1 =================
            wt1 = {}
            st1 = {"g": 0}
            ag_emitted = [False] * npieces
            ag_pending = []      # (piece, ready_at_call)
            ag_ready = []

            def alloc_agg():
                return apool.tile([128, SLAB], f32, tag="agg", name="agg")

            def l1_call_end(ncall, last_w):
                if ncall == 1 and not stage2[0]:
                    load_stage2()
                close_groups(1, z1, wt1, last_w // 16 - 1, st1)
                # emit pending AllGathers a few calls after their h piece
                # completed, so their sem waits never head-block the Pool
                # queue ahead of gather dispatches
                while ag_pending and ncall >= ag_pending[0][1] + AG_DELAY:
                    emit_ag(ag_pending.pop(0)[0])
                ag_ready.append(ncall)

            emit_stream(st["bud1"], st["kinds1"], 0, ix1, 0, xpair_d[:],
                        wt1, alloc_agg, l1_call_end)
            close_groups(1, z1, wt1, 6, st1)
            while ag_pending:
                emit_ag(ag_pending.pop(0)[0])

            # ================= layer 2 =================
            kbase = nch1
            ix_off = 0
            for p in range(npieces):
                wt2 = {}
                st2 = {"g": 0}
                lastp = (p == npieces - 1)

                def call_end(ncall, last_w, _p=p, _wt=wt2, _st=st2,
                             _lp=lastp):
                    if (_p == 0 and ncall == AG_AFTER_P1_CALLS
                            and not ag_emitted[npieces - 1]):
                        # last h piece is complete by now; emitting here
                        # keeps its sem wait from blocking the Pool SEQ
                        # ahead of the piece-0 gather dispatches
                        emit_ag(npieces - 1)
                        ag_emitted[npieces - 1] = True
                    close_groups(2, z2, _wt, last_w // 16 - 1, _st,
                                 add=(_p > 0), run_slabs=_lp)

                if p == npieces - 1 and not ag_emitted[p]:
                    emit_ag(p)          # safety: piece-0 stream was short
                    ag_emitted[p] = True
                base = h_table[p][:]
                tab = AP(base.tensor, 0, [[128, 8 * np_p[p]], [1, 128]])
                emit_stream(st["bud2"][p], st["kinds2"][p], kbase, ix2,
                            ix_off, tab, wt2, alloc_agg, call_end)
                close_groups(2, z2, wt2, 6, st2, add=(p > 0),
                             run_slabs=lastp)
                kbase += st["nch2"][p]
                ix_off += st["S2"][p] // 16
            if debug:
                nc.sync.dma_start(dbg_z1_d[:], z1[:].bitcast(f32))
                nc.sync.dma_start(dbg_z2_d[:], z2[:].bitcast(f32))

    nc.compile()
    return nc


def _make_in_maps(features, W_self1, W_neigh1, b1, W_self2, W_neigh2, b2,
                  st, pc):
    feat = np.ascontiguousarray(features, dtype=np.float32)
    xpair = feat.astype(np.float16).reshape(N_NODES // 2, 128)
    w1c = np.vstack([W_self1, W_neigh1]).astype(np.float32)
    w2c = np.vstack([W_self2, W_neigh2]).astype(np.float32)
    b1c = np.asarray(b1, np.float32).reshape(-1, 1)
    b2c = np.asarray(b2, np.float32).reshape(-1, 1)
    iota = np.tile(np.arange(128, dtype=np.float16), (128, 1))
    ident = np.eye(IN_F, dtype=np.float32)
    NW_ = NW
    in_maps = []
    for c in range(M_CORES):
        sl = slice(c * NPC, (c + 1) * NPC)
        ivd = np.ascontiguousarray(
            np.tile(pc["invdeg"][sl], (64, 1)))
        in_maps.append({
            "xpair": xpair,
            "xT": np.ascontiguousarray(feat[sl].T),
            "w1c": w1c, "w2c": w2c, "b1c": b1c, "b2c": b2c,
            "iota": iota, "ident": ident,
            "invd": ivd,
            "dstrel": pc["drt"][c],
            "idx1": pc["idx1"][c],
            "idx2": pc["idx2"][c],
        })
    return in_maps


_TRACE_RESULT = {}


def kernel(features, W_self1, W_neigh1, b1, W_self2, W_neigh2, b2, src, dst,
           _trace=False):
    from concourse.bass_utils import run_bass_kernel_spmd

    features = np.asarray(features, np.float32)
    src = np.asarray(src, np.int64)
    dst = np.asarray(dst, np.int64)

    # relabel nodes to flatten per-window in-degree sums (less padding)
    deg = np.bincount(dst, minlength=N_NODES)
    perm = _balance_permutation(deg)
    inv = np.empty(N_NODES, np.int64)
    inv[perm] = np.arange(N_NODES)

    st, pc = _prep(inv[src], inv[dst])
    nc = _build_bass(st)
    in_maps = _make_in_maps(features[perm], W_self1, W_neigh1, b1,
                            W_self2, W_neigh2, b2, st, pc)
    est_ns = None
    if _trace:
        try:
            from concourse.timeline_sim import TimelineSim
            ts = TimelineSim(nc, no_exec=True)
            ts.simulate()
            est_ns = int(ts.time)
        except Exception:
            import traceback
            traceback.print_exc()
    res = run_bass_kernel_spmd(nc, in_maps, core_ids=list(range(M_CORES)),
                               trace=False)
    exec_ns = res.exec_time_ns if res.exec_time_ns is not None else est_ns
    _TRACE_RESULT.clear()
    _TRACE_RESULT.update(dict(exec_time_ns=exec_ns,
                              trace=res.instructions_and_trace))
    out = np.concatenate([r["out"].T for r in res.results], axis=0)
    res_full = np.empty_like(out)
    res_full[perm] = out           # un-permute rows to original node ids
    return res_full.astype(np.float32)


# revision 9
# speedup vs baseline: 1.0367x; 1.0367x over previous
"""Trainium2 Bass kernel for a 2-layer mean-aggregation GraphSAGE GNN.

Strategy (8 NeuronCores, SPMD single program):
  - Destination nodes sharded contiguously across cores (6250/core), with
    a host-side node relabeling that (a) balances total in-degree across
    cores and (b) packs each 64-dst PSUM window to <= 1024 incoming edges
    (greedy + swap repair), so the max-over-cores window budgets hit the
    128-slot quantization exactly (L1 stream = 100096 slots vs 112512
    unbalanced).
  - Features / hidden activations stored as FP16 PAIR tables ([n/2, 128]
    rows = two consecutive node rows = 256B, the dma_gather granularity),
    so gather indices are pair ids < 32768 (int16) and no A/B table split
    is needed.  Parity (which pair half) is handled per chunk: where the
    even/odd split costs no padding the slots are parity-sorted and each
    chunk takes ONE fp16 matmul with the matching lhsT half; otherwise
    parity folds into the dstrel (+64) compared against a width-128 iota
    with two matmuls per chunk.
  - One-hots are built per chunk with tensor_scalar(is_equal) in fp16 —
    all-SBUF 2-byte operands hit the DVE 4x perf mode (~93ns/chunk).
    Mean division is applied per 512-col slab after the psum closes.
  - The gather/onehot/matmul stream is software-pipelined (front-end 2
    calls ahead), so close-op stalls at the DVE queue head never starve
    the PE; input loads are staged in three waves so the DMA-engine FIFO
    head stays clear for the gathers feeding the first AllGather.
  - PSUM: two windows per [128, 512] bank tile ((w%2)*64 partition half,
    ((w//2)%8)*64 column), rolling groups of 16 windows; each stream
    closes a group with ONE strided ACT copy (or DVE add for later
    layer-2 pieces) per partition half — accumulation groups stay
    contiguous per stream (required: split start/stop groups corrupt).
  - h exchange: 3 AllGathers over fp16 pair-row pieces of the local node
    range (boundaries chosen per input from candidates, slab-aligned),
    emitted a few calls after their h piece completes so their sem waits
    never head-block the Pool queue; layer-2 slot streams are split by
    src piece so gathers fire as each AllGather lands.
  - Projections via float32r matmuls over 512-col slabs (1 cycle/row).
  - Final [32, 6250] per-core output is transposed/concatenated and
    un-permuted on host.
"""

import os
import sys

import numpy as np

for _p in ("/opt/trn_rl_repo", "/root/.axon_site/_ro/trn_rl_repo"):
    if os.path.isdir(_p) and _p not in sys.path:
        sys.path.append(_p)

# ---- problem constants (hardcoded per harness contract) ----
N_NODES = 50000
N_EDGES = 800000
IN_F = 64
HID = 64
OUT_C = 32
M_CORES = 8
NPC = N_NODES // M_CORES     # 6250
WIN = 64
NW = -(-NPC // WIN)          # 98
GB = 2048                    # slots per dma_gather call
SLAB = 512                   # projection slab (psum bank cols)
# piece boundaries must be SLAB multiples (h DMAs never straddle pieces)
PIECE_CANDS = [(2048, 4096), (1536, 4096), (2048, 4608), (2560, 4608),
               (2048, 3584), (3072,), (3584,), (2560,)]
AG_AFTER_P1_CALLS = 8        # issue last AllGather after this many L2p1 calls
AG_DELAY = int(os.environ.get('AG_DELAY', 3))   # calls between h-piece completion and its AG
NS_PER_SLOT = 1.4225e-3      # µs, gather DMA cost per slot (cost model)


def _round_up(x, k):
    return (x + k - 1) // k * k


def _balance_permutation(deg):
    """Relabel nodes so per-core and per-window in-degree sums are flat,
    minimizing the round-128 padding of the max-over-cores window budgets.
    Returns perm (new position -> original node id)."""
    import heapq

    order = np.argsort(-deg, kind="stable")
    # 1) balance total in-degree across cores (capacity NPC each)
    heap = [(0, c) for c in range(M_CORES)]
    heapq.heapify(heap)
    cap = [NPC] * M_CORES
    core_nodes = [[] for _ in range(M_CORES)]
    for v in order:
        picked = []
        while True:
            s, c = heapq.heappop(heap)
            if cap[c] > 0:
                break
            picked.append((s, c))
        core_nodes[c].append(v)
        cap[c] -= 1
        heapq.heappush(heap, (s + int(deg[v]), c))
        for it in picked:
            heapq.heappush(heap, it)

    # 2) within each core, pack windows toward exact 1024-edge targets
    perm = np.empty(N_NODES, np.int64)
    lastcap = NPC - (NW - 1) * WIN
    for c in range(M_CORES):
        nodes = core_nodes[c]                     # desc by degree
        tot = int(deg[nodes].sum())
        t_last = max(tot - (NW - 1) * 8 * 128, 8 * int(deg[nodes[0]]))
        # heap of (-slack, w); assign heaviest node to max-slack window
        caps = [WIN] * (NW - 1) + [lastcap]
        tgts = [8 * 128] * (NW - 1) + [t_last]
        sums = [0] * NW
        cnts = [0] * NW
        heap2 = [(-tgts[w], w) for w in range(NW)]
        heapq.heapify(heap2)
        wassign = [[] for _ in range(NW)]
        for v in nodes:
            picked = []
            while True:
                negslack, w = heapq.heappop(heap2)
                if cnts[w] < caps[w]:
                    break
                picked.append((negslack, w))
            wassign[w].append(v)
            cnts[w] += 1
            sums[w] += int(deg[v])
            heapq.heappush(heap2, (sums[w] - tgts[w], w))
            for it in picked:
                heapq.heappush(heap2, it)
        # 3) repair pass: swap nodes between windows so no full window
        # exceeds the 1024-edge (8-chunk) budget bin; the short last
        # window may absorb up to its 768 bin.
        BIN = 8 * 128
        from collections import defaultdict
        wdeg = [defaultdict(list) for _ in range(NW)]   # deg -> [nodes]
        for w in range(NW):
            for v in wassign[w]:
                wdeg[w][int(deg[v])].append(v)

        def room(u):
            cap_u = BIN if u < NW - 1 else 768
            return cap_u - sums[u]

        for w in range(NW - 1):
            guard = 0
            while sums[w] > BIN and guard < 200:
                guard += 1
                need = sums[w] - BIN
                done = False
                for da in sorted(wdeg[w], reverse=True):
                    if not wdeg[w][da]:
                        continue
                    for k in range(min(need, da - 1), 0, -1):
                        db = da - k
                        for u in range(NW):
                            if (u == w or room(u) < k
                                    or not wdeg[u].get(db)):
                                continue
                            a = wdeg[w][da].pop()
                            b = wdeg[u][db].pop()
                            wdeg[w][db].append(b)
                            wdeg[u][da].append(a)
                            ia = wassign[w].index(a)
                            ib = wassign[u].index(b)
                            wassign[w][ia] = b
                            wassign[u][ib] = a
                            sums[w] -= k
                            sums[u] += k
                            done = True
                            break
                        if done:
                            break
                    if done:
                        break
                if not done:
                    break

        # 4) parity balance: within each window, order nodes so the
        # even-position half and odd-position half carry (<= bud/2) each
        # -> layer-1 chunks are parity-pure with zero extra padding
        for w in range(NW):
            nodes = sorted(wassign[w], key=lambda v: -deg[v])
            hcap = _round_up(max(sums[w], 1), 128) // 2
            bins = [[], []]
            bsum = [0, 0]
            bcap = [(len(nodes) + 1) // 2, len(nodes) // 2]
            for v in nodes:
                t = 0 if (bsum[0] <= bsum[1] and len(bins[0]) < bcap[0]) \
                    else (1 if len(bins[1]) < bcap[1] else 0)
                bins[t].append(v)
                bsum[t] += int(deg[v])
            for _ in range(64):
                hi = 0 if bsum[0] >= bsum[1] else 1
                excess = bsum[hi] - hcap
                if excess <= 0:
                    break
                done = False
                for a in sorted(bins[hi], key=lambda v: -deg[v]):
                    for b in sorted(bins[1 - hi], key=lambda v: deg[v]):
                        k = int(deg[a]) - int(deg[b])
                        if 1 <= k <= excess and bsum[1 - hi] + k <= hcap:
                            bins[hi].remove(a)
                            bins[1 - hi].remove(b)
                            bins[hi].append(b)
                            bins[1 - hi].append(a)
                            bsum[hi] -= k
                            bsum[1 - hi] += k
                            done = True
                            break
                    if done:
                        break
                if not done:
                    break
            merged = []
            for i in range(len(nodes)):
                merged.append(bins[i % 2][i // 2])
            wassign[w] = merged

        pos = c * NPC
        for w in range(NW):
            for v in wassign[w]:
                perm[pos] = v
                pos += 1
        assert pos == (c + 1) * NPC
    return perm


def _stream_layout(core, w, par, rel, gidx):
    """Parity-sorted padded slot streams for one gather stream.

    Within each window segment: even-parity slots first, padded to the
    shared boundary E_w = max-over-cores even count; odd slots after.
    Chunks below/above the boundary are parity-pure and take ONE matmul
    with the matching lhsT half; the (single) straddling chunk is mixed
    and uses the parity-folded dstrel + two matmuls.

    Returns (budgets_chunks[NW], kinds per chunk 'E'/'O'/'M',
             idx [M, S], drl [M, S], S).
    """
    ce = np.zeros((M_CORES, NW), np.int64)
    co = np.zeros((M_CORES, NW), np.int64)
    np.add.at(ce, (core[par == 0], w[par == 0]), 1)
    np.add.at(co, (core[par == 1], w[par == 1]), 1)
    E = ce.max(axis=0)
    O = co.max(axis=0)
    J = (ce + co).max(axis=0)
    bud = np.maximum(_round_up(J, 128), 128)
    # sorted mode only where the parity split costs no extra padding;
    # otherwise fold parity into dstrel (all chunks mixed)
    smode = _round_up(E + O, 128) <= bud
    # even/odd boundary: chunk-aligned when the odds still fit
    B = np.where(_round_up(E, 128) + O <= bud, _round_up(E, 128), E)
    B = np.where(smode, B, 0)
    seg_off = np.concatenate([[0], np.cumsum(bud)])
    S = int(seg_off[-1])

    # chunk kinds (shared across cores)
    kinds = []
    for wi in range(NW):
        nch = int(bud[wi]) // 128
        for j in range(nch):
            lo, hi = j * 128, (j + 1) * 128
            if not smode[wi]:
                kinds.append("M")
            elif hi <= B[wi]:
                kinds.append("E")
            elif lo >= B[wi]:
                kinds.append("O")
            else:
                kinds.append("M")

    # slot positions: sorted windows rank evens from seg start, odds
    # from B; folded windows rank jointly
    pcls = np.where(smode[w], par, 0)
    key = (core * NW + w) * 2 + pcls
    order = np.argsort(key, kind="stable")
    ks = key[order]
    grp_start = np.searchsorted(ks, np.arange(M_CORES * NW * 2), side="left")
    ranks = np.arange(len(ks)) - grp_start[ks]
    wo = w[order]
    po = par[order]
    pos = seg_off[wo] + np.where(smode[wo] & (po == 1), B[wo], 0) + ranks

    # dstrel: fold +64*parity for folded windows and mixed-chunk odds
    ch_in_seg = (pos - seg_off[wo]) // 128
    mixed = (ch_in_seg * 128 < B[wo]) & ((ch_in_seg + 1) * 128 > B[wo])
    fold = (~smode[wo]) | (mixed & (po == 1))
    drl_o = np.where(fold & (po == 1), rel[order] + WIN,
                     rel[order]).astype(np.float32)

    idx_buf = np.zeros((M_CORES, S), np.int64)
    drl_buf = np.full((M_CORES, S), -1.0, np.float32)
    idx_buf[core[order], pos] = gidx[order]
    drl_buf[core[order], pos] = drl_o
    return [int(b) // 128 for b in bud], kinds, idx_buf, drl_buf, S


def _wrap_idx(streams):
    """[M, S] int -> per-core [128, S/16] int16 gather-index layout."""
    res = []
    for c in range(M_CORES):
        a = streams[c].astype(np.int16).reshape(-1, 16).T
        res.append(np.ascontiguousarray(np.tile(a, (8, 1))))
    return res


def _prep(src, dst):
    deg = np.bincount(dst, minlength=N_NODES).astype(np.int64)
    invdeg = (1.0 / np.maximum(deg, 1.0)).astype(np.float32)

    core = dst // NPC
    dloc = dst % NPC
    w_e = dloc // WIN
    par = (src & 1).astype(np.int64)
    drel = (dloc % WIN + WIN * par).astype(np.float32)
    sloc = src % NPC
    scor = src // NPC

    # ---- L1: single stream, gather from global x pair table ----
    rel = (dloc % WIN).astype(np.float32)
    bud1, kinds1, idx1_buf, drl1_buf, S1 = _stream_layout(
        core, w_e, par, rel, src >> 1)

    # ---- choose L2 piece boundaries (min estimated critical chain) ----
    l1_dma = S1 * NS_PER_SLOT            # µs
    best = None
    for bounds in PIECE_CANDS:
        offs = [0] + list(bounds) + [NPC]
        piece_slots = []
        tot = 0
        for p in range(len(offs) - 1):
            sel = (sloc >= offs[p]) & (sloc < offs[p + 1])
            c = np.zeros((M_CORES, NW), np.int64)
            np.add.at(c, (core[sel], w_e[sel]), 1)
            b = _round_up(c.max(axis=0), 128)
            if p == 0:
                b = np.maximum(b, 128)
            piece_slots.append(int(b.sum()))
            tot += int(b.sum())
        # chain: AGs serialize on the collective resource; last piece's
        # gathers wait for its AG
        t = 0.0
        for p in range(len(offs) - 1):
            ready = offs[p + 1] / NPC * l1_dma + 18.0
            nodes = offs[p + 1] - offs[p]
            ag = 15.0 + nodes * 8 * 128 / 40e3
            t = max(t, ready) + ag
        span = max(t + piece_slots[-1] * NS_PER_SLOT + 12.0,
                   (S1 + tot) * NS_PER_SLOT + 25.0)
        if best is None or span < best[0]:
            best = (span, offs)
    if os.environ.get("FORCE_OFFS"):
        best = (0, [0] + [int(x) for x in
                          os.environ["FORCE_OFFS"].split(",")] + [NPC])
    offs = best[1]
    npieces = len(offs) - 1
    assert all(o % SLAB == 0 for o in offs[1:-1])

    # ---- L2: one stream per piece, gather from h pair-piece tables ----
    bud2, kinds2, S2 = [], [], []
    idx2_bufs, drl2_bufs = [], []
    for p in range(npieces):
        np_p = (offs[p + 1] - offs[p]) // 2          # pairs/core in piece
        sel = (sloc >= offs[p]) & (sloc < offs[p + 1])
        gidx = scor[sel] * np_p + (sloc[sel] - offs[p]) // 2
        assert gidx.max() < 8 * np_p <= 32768
        b, kn, ib, db, S = _stream_layout(
            core[sel], w_e[sel], par[sel], rel[sel], gidx)
        bud2.append(b)
        kinds2.append(kn)
        idx2_bufs.append(ib)
        drl2_bufs.append(db)
        S2.append(S)

    st = dict(
        offs=offs, npieces=npieces,
        bud1=bud1, bud2=bud2,
        kinds1=kinds1, kinds2=kinds2,
        S1=S1, S2=S2,
        nch1=S1 // 128, nch2=[s // 128 for s in S2],
    )

    # per-core drt: [128, nch1 + sum(nch2)] f32 (L1 cols then L2 pieces)
    drt = []
    for c in range(M_CORES):
        cols = [drl1_buf[c].reshape(-1, 128).T]
        for p in range(npieces):
            cols.append(drl2_bufs[p][c].reshape(-1, 128).T)
        drt.append(np.ascontiguousarray(np.concatenate(cols, axis=1)))

    idx1 = _wrap_idx(idx1_buf)
    idx2 = []
    for c in range(M_CORES):
        blocks = [idx2_bufs[p][c] for p in range(npieces)]
        flat = np.concatenate(blocks)
        a = flat.astype(np.int16).reshape(-1, 16).T
        idx2.append(np.ascontiguousarray(np.tile(a, (8, 1))))

    pc = dict(drt=drt, idx1=idx1, idx2=idx2, invdeg=invdeg)
    return st, pc


def _build_bass(st, debug=False):
    import concourse.bass as bass
    import concourse.mybir as mybir
    import concourse.tile as tile
    from concourse.ap import AP
    from concourse import bacc, library_config

    f32 = mybir.dt.float32
    f32r = mybir.dt.float32r
    fp16 = mybir.dt.float16
    i16 = mybir.dt.int16
    AF = mybir.ActivationFunctionType
    OP = mybir.AluOpType

    offs = st["offs"]
    npieces = st["npieces"]
    nch1 = st["nch1"]
    nch2 = st["nch2"]
    NCH = nch1 + sum(nch2)
    np_p = [(offs[p + 1] - offs[p]) // 2 for p in range(npieces)]

    nc = bacc.Bacc(None, target_bir_lowering=False)

    xpair_d = nc.dram_tensor("xpair", [N_NODES // 2, 128], fp16,
                             kind="ExternalInput")
    xT_d = nc.dram_tensor("xT", [IN_F, NPC], f32r, kind="ExternalInput")
    w1c_d = nc.dram_tensor("w1c", [2 * IN_F, HID], f32r, kind="ExternalInput")
    w2c_d = nc.dram_tensor("w2c", [2 * HID, OUT_C], f32r, kind="ExternalInput")
    b1_d = nc.dram_tensor("b1c", [HID, 1], f32, kind="ExternalInput")
    b2_d = nc.dram_tensor("b2c", [OUT_C, 1], f32, kind="ExternalInput")
    iota_d = nc.dram_tensor("iota", [128, 128], fp16, kind="ExternalInput")
    ident_d = nc.dram_tensor("ident", [IN_F, IN_F], f32, kind="ExternalInput")
    invd_d = nc.dram_tensor("invd", [64, NPC], f32, kind="ExternalInput")
    drel_d = nc.dram_tensor("dstrel", [128, NCH], f32, kind="ExternalInput")
    idx1_d = nc.dram_tensor("idx1", [128, st["S1"] // 16], i16,
                            kind="ExternalInput")
    idx2_d = nc.dram_tensor("idx2", [128, sum(st["S2"]) // 16], i16,
                            kind="ExternalInput")
    out_d = nc.dram_tensor("out", [OUT_C, NPC], f32, kind="ExternalOutput")
    scratch_d = nc.dram_tensor("scratch", [1, 64], fp16)
    if debug:
        dbg_z1_d = nc.dram_tensor("dbg_z1", [128, NPC], f32,
                                  kind="ExternalOutput")
        dbg_z2_d = nc.dram_tensor("dbg_z2", [128, NPC], f32,
                                  kind="ExternalOutput")

    h_shard = [nc.dram_tensor(f"h_shard_{p}", [2 * np_p[p], HID], fp16)
               for p in range(npieces)]
    h_table = [nc.dram_tensor(f"h_table_{p}", [2 * 8 * np_p[p], HID], fp16,
                              addr_space="Shared")
               for p in range(npieces)]

    with tile.TileContext(nc) as tc:
        nc.gpsimd.load_library(library_config.mlp)
        with (
            tc.tile_pool(name="const", bufs=1) as cpool,
            tc.tile_pool(name="gath", bufs=int(os.environ.get("GBUFS", 3))) as gpool,
            tc.tile_pool(name="oh", bufs=int(os.environ.get("GBUFS", 3))) as ohpool,
            tc.tile_pool(name="hsb", bufs=12) as hpool,
            tc.tile_pool(name="osl", bufs=3) as opool,
            tc.tile_pool(name="agg", bufs=6, space="PSUM") as apool,
            tc.tile_pool(name="msc", bufs=1, space="PSUM") as mpool,
            tc.tile_pool(name="prj", bufs=1, space="PSUM") as jpool,
        ):
            # ---- persistent SBUF ----
            z1 = cpool.tile([2 * IN_F, NPC], f32r, tag="z1")
            z2 = cpool.tile([2 * HID, NPC], f32r, tag="z2")
            w1t = cpool.tile([2 * IN_F, HID], f32r, tag="w1t")
            w2t = cpool.tile([2 * HID, OUT_C], f32r, tag="w2t")
            b1t = cpool.tile([HID, 1], f32, tag="b1t")
            b2t = cpool.tile([OUT_C, 1], f32, tag="b2t")
            iot = cpool.tile([128, 128], fp16, tag="iot")
            idt = cpool.tile([IN_F, IN_F], f32, tag="idt")
            ivt = cpool.tile([128, NPC], f32, tag="ivt")
            drt = cpool.tile([128, NCH], f32, tag="drt")
            ix1 = cpool.tile([128, st["S1"] // 16], i16, tag="ix1")
            ix2 = cpool.tile([128, sum(st["S2"]) // 16], i16, tag="ix2")

            # load order matters: the DMA engines are FIFO, so the first
            # gather call queues behind whatever consts precede it.  Load
            # only the L1-stream-critical prefix first (idx/dstrel split
            # so subtile deps release the first gather early); defer the
            # L2-only loads behind the first h write (SP queue blocks on
            # it, staggering them off the head of the DMA FIFO).
            # staged input loads: the DMA engines are FIFO, so anything
            # loaded before the gathers a piece-1 AllGather depends on
            # delays the whole collective chain.  stage0 = minimal stream
            # prefix; stage2 (after call 1, held back by a blocker DMA
            # reading call-1's onehot) = what layer-1 slabs 0..7 need;
            # stage3 (after piece-1's h is written) = everything else.
            IX1H = min(2560 // 16 * 16, st["S1"] // 16)
            DRTH = min(320, NCH)
            XTH = min(4096, NPC)
            nc.sync.dma_start(ix1[:, 0:IX1H], idx1_d[:, 0:IX1H])
            nc.sync.dma_start(iot[:], iota_d[:])
            nc.sync.dma_start(drt[:, 0:DRTH], drel_d[:, 0:DRTH])

            last_oh = [None]
            stage2 = [False]

            def load_stage2():
                nc.sync.dma_start(scratch_d[0:1, :],
                                  last_oh[0][0:1, 0, 0:64])
                nc.sync.dma_start(w1t[:], w1c_d[:])
                nc.sync.dma_start(b1t[:], b1_d[:])
                nc.sync.dma_start(z1[0:IN_F, 0:XTH], xT_d[:, 0:XTH])
                nc.sync.dma_start(ivt[64:128, 0:XTH], invd_d[:, 0:XTH])
                nc.sync.dma_start(idt[:], ident_d[:])
                stage2[0] = True

            deferred = [False]

            def load_deferred():
                nc.sync.dma_start(ix1[:, IX1H:], idx1_d[:, IX1H:])
                nc.sync.dma_start(drt[:, DRTH:], drel_d[:, DRTH:])
                nc.sync.dma_start(z1[0:IN_F, XTH:], xT_d[:, XTH:])
                nc.sync.dma_start(ivt[64:128, XTH:], invd_d[:, XTH:])
                nc.sync.dma_start(ix2[:], idx2_d[:])
                nc.sync.dma_start(w2t[:], w2c_d[:])
                nc.sync.dma_start(b2t[:], b2_d[:])
                deferred[0] = True

            # misc psum bank: [:, 0:64] = windows 96/97, [:, 64+64j] = L1
            # transpose slots
            misc = mpool.tile([128, SLAB], f32, tag="misc")

            # ---------------- shared machinery ----------------
            def win_slice(wtiles, w, wn):
                t = w // 2
                if t == 48:
                    tl = misc
                    col = 0
                else:
                    tl = wtiles[t // 8]
                    col = (t % 8) * 64
                ph = (w % 2) * 64
                return tl[ph: ph + 64, col: col + wn]

            def emit_stream(chunks_per_win, kinds, kbase, ixt, ix_off,
                            table_ap, wtiles, alloc_group, on_call_end):
                """Issue gather/onehot/matmul stream, software-pipelined:
                gathers + onehots run PIPE calls ahead of the matmuls, so
                a close-op stall at the DVE queue head never starves the
                PE.  Each window's psum accumulation group is contiguous
                WITHIN this stream.  Parity-pure chunks ('E'/'O') take one
                matmul with the matching lhsT half; mixed chunks use the
                folded dstrel + two."""
                chlist = []
                for w in range(NW):
                    for j in range(chunks_per_win[w]):
                        chlist.append((w, j))
                calls = []
                k = 0
                while k < len(chlist):
                    nb = min(GB // 128, len(chlist) - k)
                    calls.append((k, nb))
                    k += nb
                PIPE = int(os.environ.get('PIPE', 1))
                tiles = {}

                def front(ci):
                    k, nb = calls[ci]
                    b0 = k * 128
                    g = gpool.tile([128, GB // 128, 128], fp16, tag="g",
                                   name="g")
                    nc.gpsimd.dma_gather(
                        out_ap=g[:, 0:nb, :],
                        in_ap=table_ap,
                        idxs_ap=ixt[:, ix_off + b0 // 16:
                                    ix_off + b0 // 16 + nb * 8],
                        num_idxs=nb * 128,
                        num_idxs_reg=nb * 128,
                        elem_size=128,
                        single_packet=False,
                    )
                    oh = ohpool.tile([128, GB // 128, 128], fp16, tag="oh",
                                     name="oh")
                    last_oh[0] = oh
                    for col in range(nb):
                        kind = kinds[k + col]
                        ohw = 128 if kind == "M" else 64
                        # pure onehot (iota == dstrel), fp16, 4x DVE mode
                        nc.vector.tensor_scalar(
                            out=oh[:, col, 0:ohw],
                            in0=iot[:, 0:ohw],
                            scalar1=drt[:, kbase + k + col:
                                        kbase + k + col + 1],
                            scalar2=None,
                            op0=OP.is_equal,
                        )
                    tiles[ci] = (g, oh)

                def back(ci):
                    k, nb = calls[ci]
                    g, oh = tiles.pop(ci)
                    for col in range(nb):
                        w, j = chlist[k + col]
                        kind = kinds[k + col]
                        wn = min(WIN, NPC - w * WIN)
                        if (w // 2) != 48 and (w // 16) not in wtiles:
                            wtiles[w // 16] = alloc_group()
                        ps = win_slice(wtiles, w, wn)
                        first = (j == 0)
                        last = (j == chunks_per_win[w] - 1)
                        if kind == "M":
                            nc.tensor.matmul(
                                ps, g[:, col, 0:64], oh[:, col, 0:wn],
                                start=first, stop=False)
                            nc.tensor.matmul(
                                ps, g[:, col, 64:128],
                                oh[:, col, 64:64 + wn],
                                start=False, stop=last)
                        else:
                            half = slice(0, 64) if kind == "E" \
                                else slice(64, 128)
                            nc.tensor.matmul(
                                ps, g[:, col, half], oh[:, col, 0:wn],
                                start=first, stop=last)
                    on_call_end(ci + 1, chlist[k + nb - 1][0])

                for ci in range(len(calls)):
                    front(ci)
                    if ci >= PIPE:
                        back(ci - PIPE)
                for ci in range(max(0, len(calls) - PIPE), len(calls)):
                    back(ci)

            def close_groups(layer, z, wtiles, upto_g, state, add=False,
                             run_slabs=True):
                """Close whole 16-window psum groups <= upto_g: one strided
                ACT copy (or DVE add) per (bank tile, partition half)
                moves 8 windows at once; then run slab completions."""
                ng = 7  # groups 0..5 full tiles, group 6 = windows 96/97
                while state["g"] <= min(upto_g, ng - 1):
                    g = state["g"]
                    if g < 6:
                        tl = wtiles[g]
                        for ph in (0, 64):
                            # even (ph=0) / odd (ph=64) windows of group
                            zb = z[HID:, (16 * g + ph // 64) * WIN:
                                   (16 * g + ph // 64) * WIN + WIN]
                            zsl = AP(zb.tensor, zb.offset,
                                     [zb.ap[0], [128, 8], [1, WIN]])
                            pb = tl[ph: ph + 64, 0:SLAB]
                            psl = AP(pb.tensor, pb.offset,
                                     [pb.ap[0], [WIN, 8], [1, WIN]])
                            if not add:
                                nc.scalar.copy(zsl, psl)
                            else:
                                nc.vector.scalar_tensor_tensor(
                                    out=zsl, in0=psl, scalar=1.0, in1=zsl,
                                    op0=OP.mult, op1=OP.add)
                    else:
                        for w in (96, 97):
                            wn = min(WIN, NPC - w * WIN)
                            zsl = z[HID:, w * WIN: w * WIN + wn]
                            ps = win_slice(wtiles, w, wn)
                            if not add:
                                nc.scalar.copy(zsl, ps)
                            else:
                                nc.vector.scalar_tensor_tensor(
                                    out=zsl, in0=ps, scalar=1.0, in1=zsl,
                                    op0=OP.mult, op1=OP.add)
                    state["g"] += 1
                    if run_slabs:
                        for s in (2 * g, 2 * g + 1):
                            if s * SLAB < NPC:
                                finish_slab(layer, z, s)

            def finish_slab(layer, z, s):
                a, b = s * SLAB, min((s + 1) * SLAB, NPC)
                cols = b - a
                # mean division (invdeg folded out of the onehots)
                nc.vector.scalar_tensor_tensor(
                    out=z[HID:, a:b], in0=z[HID:, a:b], scalar=1.0,
                    in1=ivt[64:128, a:b].bitcast(f32r),
                    op0=OP.mult, op1=OP.mult)
                if layer == 1:
                    p1 = jpool.tile([HID, SLAB], f32, tag="prj",
                                    name="prj")
                    nc.tensor.matmul(p1[:, :cols], w1t[:], z[:, a:b],
                                     start=True, stop=True)
                    nc.scalar.activation(z2[0:HID, a:b], p1[:, :cols],
                                         AF.Relu, bias=b1t[:, 0:1])
                    for j4 in range(-(-cols // 128)):
                        ca = a + j4 * 128
                        cb = min(ca + 128, b)
                        cc = cb - ca
                        tp = misc[:, 64 + (j4 % 4) * 64:
                                  128 + (j4 % 4) * 64]
                        nc.tensor.transpose(tp[:cc, :],
                                            z2[0:HID, ca:cb].bitcast(f32),
                                            idt[:])
                        hs = hpool.tile([128, HID], fp16, tag="hs",
                                        name="hs")
                        nc.scalar.copy(hs[:cc, :], tp[:cc, :])
                        # piece containing this chunk
                        p = next(i for i in range(npieces)
                                 if offs[i] <= ca < offs[i + 1])
                        nc.sync.dma_start(
                            h_shard[p][ca - offs[p]: cb - offs[p], :],
                            hs[:cc, :])
                    if not deferred[0] and b >= offs[1]:
                        load_deferred()
                    # queue AllGathers for completed h pieces (all but the
                    # last, which is deferred into the L2 piece-0 stream)
                    for p in range(npieces - 1):
                        if not ag_emitted[p] and b >= offs[p + 1]:
                            ag_pending.append(
                                (p, ag_ready[-1] if ag_ready else 0))
                            ag_emitted[p] = True
                else:
                    p2 = jpool.tile([HID, SLAB], f32, tag="prj",
                                    name="prj")[0:OUT_C, :]
                    nc.tensor.matmul(p2[:, :cols], w2t[:], z[:, a:b],
                                     start=True, stop=True)
                    osl = opool.tile([OUT_C, SLAB], f32, tag="osl",
                                     name="osl")
                    nc.scalar.activation(osl[:, :cols], p2[:, :cols],
                                         AF.Identity, bias=b2t[:, 0:1])
                    nc.sync.dma_start(out_d[:, a:b], osl[:, :cols])

            def emit_ag(p):
                nc.gpsimd.collective_compute(
                    "AllGather",
                    mybir.AluOpType.bypass,
                    replica_groups=[list(range(M_CORES))],
                    ins=[h_shard[p][:]],
                    outs=[h_table[p][:]],
                )

            # ================= layer 1 =================
            wt1 = {}
            st1 = {"g": 0}
            ag_emitted = [False] * npieces
            ag_pending = []      # (piece, ready_at_call)
            ag_ready = []

            def alloc_agg():
                return apool.tile([128, SLAB], f32, tag="agg", name="agg")

            def l1_call_end(ncall, last_w):
                if ncall == 1 and not stage2[0]:
                    load_stage2()
                close_groups(1, z1, wt1, last_w // 16 - 1, st1)
                # emit pending AllGathers a few calls after their h piece
                # completed, so their sem waits never head-block the Pool
                # queue ahead of gather dispatches
                while ag_pending and ncall >= ag_pending[0][1] + AG_DELAY:
                    emit_ag(ag_pending.pop(0)[0])
                ag_ready.append(ncall)

            emit_stream(st["bud1"], st["kinds1"], 0, ix1, 0, xpair_d[:],
                        wt1, alloc_agg, l1_call_end)
            close_groups(1, z1, wt1, 6, st1)
            while ag_pending:
                emit_ag(ag_pending.pop(0)[0])

            # ================= layer 2 =================
            kbase = nch1
            ix_off = 0
            for p in range(npieces):
                wt2 = {}
                st2 = {"g": 0}
                lastp = (p == npieces - 1)

                def call_end(ncall, last_w, _p=p, _wt=wt2, _st=st2,
                             _lp=lastp):
                    if (_p == 0 and ncall == AG_AFTER_P1_CALLS
                            and not ag_emitted[npieces - 1]):
                        # last h piece is complete by now; emitting here
                        # keeps its sem wait from blocking the Pool SEQ
                        # ahead of the piece-0 gather dispatches
                        emit_ag(npieces - 1)
                        ag_emitted[npieces - 1] = True
                    close_groups(2, z2, _wt, last_w // 16 - 1, _st,
                                 add=(_p > 0), run_slabs=_lp)

                if p == npieces - 1 and not ag_emitted[p]:
                    emit_ag(p)          # safety: piece-0 stream was short
                    ag_emitted[p] = True
                base = h_table[p][:]
                tab = AP(base.tensor, 0, [[128, 8 * np_p[p]], [1, 128]])
                emit_stream(st["bud2"][p], st["kinds2"][p], kbase, ix2,
                            ix_off, tab, wt2, alloc_agg, call_end)
                close_groups(2, z2, wt2, 6, st2, add=(p > 0),
                             run_slabs=lastp)
                kbase += st["nch2"][p]
                ix_off += st["S2"][p] // 16
            if debug:
                nc.sync.dma_start(dbg_z1_d[:], z1[:].bitcast(f32))
                nc.sync.dma_start(dbg_z2_d[:], z2[:].bitcast(f32))

    nc.compile()
    return nc


def _make_in_maps(features, W_self1, W_neigh1, b1, W_self2, W_neigh2, b2,
                  st, pc):
    feat = np.ascontiguousarray(features, dtype=np.float32)
    xpair = feat.astype(np.float16).reshape(N_NODES // 2, 128)
    w1c = np.vstack([W_self1, W_neigh1]).astype(np.float32)
    w2c = np.vstack([W_self2, W_neigh2]).astype(np.float32)
    b1c = np.asarray(b1, np.float32).reshape(-1, 1)
    b2c = np.asarray(b2, np.float32).reshape(-1, 1)
    iota = np.tile(np.arange(128, dtype=np.float16), (128, 1))
    ident = np.eye(IN_F, dtype=np.float32)
    NW_ = NW
    in_maps = []
    for c in range(M_CORES):
        sl = slice(c * NPC, (c + 1) * NPC)
        ivd = np.ascontiguousarray(
            np.tile(pc["invdeg"][sl], (64, 1)))
        in_maps.append({
            "xpair": xpair,
            "xT": np.ascontiguousarray(feat[sl].T),
            "w1c": w1c, "w2c": w2c, "b1c": b1c, "b2c": b2c,
            "iota": iota, "ident": ident,
            "invd": ivd,
            "dstrel": pc["drt"][c],
            "idx1": pc["idx1"][c],
            "idx2": pc["idx2"][c],
        })
    return in_maps


_TRACE_RESULT = {}


def kernel(features, W_self1, W_neigh1, b1, W_self2, W_neigh2, b2, src, dst,
           _trace=False):
    from concourse.bass_utils import run_bass_kernel_spmd

    features = np.asarray(features, np.float32)
    src = np.asarray(src, np.int64)
    dst = np.asarray(dst, np.int64)

    # relabel nodes to flatten per-window in-degree sums (less padding)
    deg = np.bincount(dst, minlength=N_NODES)
    perm = _balance_permutation(deg)
    inv = np.empty(N_NODES, np.int64)
    inv[perm] = np.arange(N_NODES)

    st, pc = _prep(inv[src], inv[dst])
    nc = _build_bass(st)
    in_maps = _make_in_maps(features[perm], W_self1, W_neigh1, b1,
                            W_self2, W_neigh2, b2, st, pc)
    est_ns = None
    if _trace:
        try:
            from concourse.timeline_sim import TimelineSim
            ts = TimelineSim(nc, no_exec=True)
            ts.simulate()
            est_ns = int(ts.time)
        except Exception:
            import traceback
            traceback.print_exc()
    res = run_bass_kernel_spmd(nc, in_maps, core_ids=list(range(M_CORES)),
                               trace=False)
    exec_ns = res.exec_time_ns if res.exec_time_ns is not None else est_ns
    _TRACE_RESULT.clear()
    _TRACE_RESULT.update(dict(exec_time_ns=exec_ns,
                              trace=res.instructions_and_trace))
    out = np.concatenate([r["out"].T for r in res.results], axis=0)
    res_full = np.empty_like(out)
    res_full[perm] = out           # un-permute rows to original node ids
    return res_full.astype(np.float32)


# revision 10
# speedup vs baseline: 1.0378x; 1.0010x over previous
"""Trainium2 Bass kernel for a 2-layer mean-aggregation GraphSAGE GNN.

Strategy (8 NeuronCores, SPMD single program):
  - Destination nodes sharded contiguously across cores (6250/core), with
    a host-side node relabeling that (a) balances total in-degree across
    cores and (b) packs each 64-dst PSUM window to <= 1024 incoming edges
    (greedy + swap repair), so the max-over-cores window budgets hit the
    128-slot quantization exactly (L1 stream = 100096 slots vs 112512
    unbalanced).
  - Features / hidden activations stored as FP16 PAIR tables ([n/2, 128]
    rows = two consecutive node rows = 256B, the dma_gather granularity),
    so gather indices are pair ids < 32768 (int16) and no A/B table split
    is needed.  Parity (which pair half) is handled per chunk: where the
    even/odd split costs no padding the slots are parity-sorted and each
    chunk takes ONE fp16 matmul with the matching lhsT half; otherwise
    parity folds into the dstrel (+64) compared against a width-128 iota
    with two matmuls per chunk.
  - One-hots are built per chunk with tensor_scalar(is_equal) in fp16 —
    all-SBUF 2-byte operands hit the DVE 4x perf mode (~93ns/chunk).
    Mean division is applied per 512-col slab after the psum closes.
  - The gather/onehot/matmul stream is software-pipelined (front-end 2
    calls ahead), so close-op stalls at the DVE queue head never starve
    the PE; input loads are staged in three waves so the DMA-engine FIFO
    head stays clear for the gathers feeding the first AllGather.
  - PSUM: two windows per [128, 512] bank tile ((w%2)*64 partition half,
    ((w//2)%8)*64 column), rolling groups of 16 windows; each stream
    closes a group with ONE strided ACT copy (or DVE add for later
    layer-2 pieces) per partition half — accumulation groups stay
    contiguous per stream (required: split start/stop groups corrupt).
  - h exchange: 3 AllGathers over fp16 pair-row pieces of the local node
    range (boundaries chosen per input from candidates, slab-aligned),
    emitted a few calls after their h piece completes so their sem waits
    never head-block the Pool queue; layer-2 slot streams are split by
    src piece so gathers fire as each AllGather lands.
  - Projections via float32r matmuls over 512-col slabs (1 cycle/row).
  - Final [32, 6250] per-core output is transposed/concatenated and
    un-permuted on host.
"""

import os
import sys

import numpy as np

for _p in ("/opt/trn_rl_repo", "/root/.axon_site/_ro/trn_rl_repo"):
    if os.path.isdir(_p) and _p not in sys.path:
        sys.path.append(_p)

# ---- problem constants (hardcoded per harness contract) ----
N_NODES = 50000
N_EDGES = 800000
IN_F = 64
HID = 64
OUT_C = 32
M_CORES = 8
NPC = N_NODES // M_CORES     # 6250
WIN = 64
NW = -(-NPC // WIN)          # 98
GB = 2048                    # slots per dma_gather call
SLAB = 512                   # projection slab (psum bank cols)
# piece boundaries must be SLAB multiples (h DMAs never straddle pieces)
PIECE_CANDS = [(2048, 4096), (1536, 4096), (2048, 4608), (2560, 4608),
               (2048, 3584), (3072,), (3584,), (2560,)]
AG_AFTER_P1_CALLS = 8        # issue last AllGather after this many L2p1 calls
AG_DELAY = int(os.environ.get('AG_DELAY', 3))   # calls between h-piece completion and its AG
NS_PER_SLOT = 1.4225e-3      # µs, gather DMA cost per slot (cost model)


def _round_up(x, k):
    return (x + k - 1) // k * k


def _balance_permutation(deg):
    """Relabel nodes so per-core and per-window in-degree sums are flat,
    minimizing the round-128 padding of the max-over-cores window budgets.
    Returns perm (new position -> original node id)."""
    import heapq

    order = np.argsort(-deg, kind="stable")
    # 1) balance total in-degree across cores (capacity NPC each)
    heap = [(0, c) for c in range(M_CORES)]
    heapq.heapify(heap)
    cap = [NPC] * M_CORES
    core_nodes = [[] for _ in range(M_CORES)]
    for v in order:
        picked = []
        while True:
            s, c = heapq.heappop(heap)
            if cap[c] > 0:
                break
            picked.append((s, c))
        core_nodes[c].append(v)
        cap[c] -= 1
        heapq.heappush(heap, (s + int(deg[v]), c))
        for it in picked:
            heapq.heappush(heap, it)

    # 2) within each core, pack windows toward exact 1024-edge targets
    perm = np.empty(N_NODES, np.int64)
    lastcap = NPC - (NW - 1) * WIN
    for c in range(M_CORES):
        nodes = core_nodes[c]                     # desc by degree
        tot = int(deg[nodes].sum())
        t_last = max(tot - (NW - 1) * 8 * 128, 8 * int(deg[nodes[0]]))
        # heap of (-slack, w); assign heaviest node to max-slack window
        caps = [WIN] * (NW - 1) + [lastcap]
        tgts = [8 * 128] * (NW - 1) + [t_last]
        sums = [0] * NW
        cnts = [0] * NW
        heap2 = [(-tgts[w], w) for w in range(NW)]
        heapq.heapify(heap2)
        wassign = [[] for _ in range(NW)]
        for v in nodes:
            picked = []
            while True:
                negslack, w = heapq.heappop(heap2)
                if cnts[w] < caps[w]:
                    break
                picked.append((negslack, w))
            wassign[w].append(v)
            cnts[w] += 1
            sums[w] += int(deg[v])
            heapq.heappush(heap2, (sums[w] - tgts[w], w))
            for it in picked:
                heapq.heappush(heap2, it)
        # 3) repair pass: swap nodes between windows so no full window
        # exceeds the 1024-edge (8-chunk) budget bin; the short last
        # window may absorb up to its 768 bin.
        BIN = 8 * 128
        from collections import defaultdict
        wdeg = [defaultdict(list) for _ in range(NW)]   # deg -> [nodes]
        for w in range(NW):
            for v in wassign[w]:
                wdeg[w][int(deg[v])].append(v)

        def room(u):
            cap_u = BIN if u < NW - 1 else 768
            return cap_u - sums[u]

        for w in range(NW - 1):
            guard = 0
            while sums[w] > BIN and guard < 200:
                guard += 1
                need = sums[w] - BIN
                done = False
                for da in sorted(wdeg[w], reverse=True):
                    if not wdeg[w][da]:
                        continue
                    for k in range(min(need, da - 1), 0, -1):
                        db = da - k
                        for u in range(NW):
                            if (u == w or room(u) < k
                                    or not wdeg[u].get(db)):
                                continue
                            a = wdeg[w][da].pop()
                            b = wdeg[u][db].pop()
                            wdeg[w][db].append(b)
                            wdeg[u][da].append(a)
                            ia = wassign[w].index(a)
                            ib = wassign[u].index(b)
                            wassign[w][ia] = b
                            wassign[u][ib] = a
                            sums[w] -= k
                            sums[u] += k
                            done = True
                            break
                        if done:
                            break
                    if done:
                        break
                if not done:
                    break

        # 4) parity balance: within each window, order nodes so the
        # even-position half and odd-position half carry (<= bud/2) each
        # -> layer-1 chunks are parity-pure with zero extra padding
        for w in range(NW):
            nodes = sorted(wassign[w], key=lambda v: -deg[v])
            hcap = _round_up(max(sums[w], 1), 128) // 2
            bins = [[], []]
            bsum = [0, 0]
            bcap = [(len(nodes) + 1) // 2, len(nodes) // 2]
            for v in nodes:
                t = 0 if (bsum[0] <= bsum[1] and len(bins[0]) < bcap[0]) \
                    else (1 if len(bins[1]) < bcap[1] else 0)
                bins[t].append(v)
                bsum[t] += int(deg[v])
            for _ in range(64):
                hi = 0 if bsum[0] >= bsum[1] else 1
                excess = bsum[hi] - hcap
                if excess <= 0:
                    break
                done = False
                for a in sorted(bins[hi], key=lambda v: -deg[v]):
                    for b in sorted(bins[1 - hi], key=lambda v: deg[v]):
                        k = int(deg[a]) - int(deg[b])
                        if 1 <= k <= excess and bsum[1 - hi] + k <= hcap:
                            bins[hi].remove(a)
                            bins[1 - hi].remove(b)
                            bins[hi].append(b)
                            bins[1 - hi].append(a)
                            bsum[hi] -= k
                            bsum[1 - hi] += k
                            done = True
                            break
                    if done:
                        break
                if not done:
                    break
            merged = []
            for i in range(len(nodes)):
                merged.append(bins[i % 2][i // 2])
            wassign[w] = merged

        pos = c * NPC
        for w in range(NW):
            for v in wassign[w]:
                perm[pos] = v
                pos += 1
        assert pos == (c + 1) * NPC
    return perm


def _stream_layout(core, w, par, rel, gidx):
    """Parity-sorted padded slot streams for one gather stream.

    Within each window segment: even-parity slots first, padded to the
    shared boundary E_w = max-over-cores even count; odd slots after.
    Chunks below/above the boundary are parity-pure and take ONE matmul
    with the matching lhsT half; the (single) straddling chunk is mixed
    and uses the parity-folded dstrel + two matmuls.

    Returns (budgets_chunks[NW], kinds per chunk 'E'/'O'/'M',
             idx [M, S], drl [M, S], S).
    """
    ce = np.zeros((M_CORES, NW), np.int64)
    co = np.zeros((M_CORES, NW), np.int64)
    np.add.at(ce, (core[par == 0], w[par == 0]), 1)
    np.add.at(co, (core[par == 1], w[par == 1]), 1)
    E = ce.max(axis=0)
    O = co.max(axis=0)
    J = (ce + co).max(axis=0)
    bud = np.maximum(_round_up(J, 128), 128)
    # sorted mode only where the parity split costs no extra padding;
    # otherwise fold parity into dstrel (all chunks mixed)
    smode = _round_up(E + O, 128) <= bud
    # even/odd boundary: chunk-aligned when the odds still fit
    B = np.where(_round_up(E, 128) + O <= bud, _round_up(E, 128), E)
    B = np.where(smode, B, 0)
    seg_off = np.concatenate([[0], np.cumsum(bud)])
    S = int(seg_off[-1])

    # chunk kinds (shared across cores)
    kinds = []
    for wi in range(NW):
        nch = int(bud[wi]) // 128
        for j in range(nch):
            lo, hi = j * 128, (j + 1) * 128
            if not smode[wi]:
                kinds.append("M")
            elif hi <= B[wi]:
                kinds.append("E")
            elif lo >= B[wi]:
                kinds.append("O")
            else:
                kinds.append("M")

    # slot positions: sorted windows rank evens from seg start, odds
    # from B; folded windows rank jointly
    pcls = np.where(smode[w], par, 0)
    key = (core * NW + w) * 2 + pcls
    order = np.argsort(key, kind="stable")
    ks = key[order]
    grp_start = np.searchsorted(ks, np.arange(M_CORES * NW * 2), side="left")
    ranks = np.arange(len(ks)) - grp_start[ks]
    wo = w[order]
    po = par[order]
    pos = seg_off[wo] + np.where(smode[wo] & (po == 1), B[wo], 0) + ranks

    # dstrel: fold +64*parity for folded windows and mixed-chunk odds
    ch_in_seg = (pos - seg_off[wo]) // 128
    mixed = (ch_in_seg * 128 < B[wo]) & ((ch_in_seg + 1) * 128 > B[wo])
    fold = (~smode[wo]) | (mixed & (po == 1))
    drl_o = np.where(fold & (po == 1), rel[order] + WIN,
                     rel[order]).astype(np.float32)

    idx_buf = np.zeros((M_CORES, S), np.int64)
    drl_buf = np.full((M_CORES, S), -1.0, np.float32)
    idx_buf[core[order], pos] = gidx[order]
    drl_buf[core[order], pos] = drl_o
    return [int(b) // 128 for b in bud], kinds, idx_buf, drl_buf, S


def _wrap_idx(streams):
    """[M, S] int -> per-core [128, S/16] int16 gather-index layout."""
    res = []
    for c in range(M_CORES):
        a = streams[c].astype(np.int16).reshape(-1, 16).T
        res.append(np.ascontiguousarray(np.tile(a, (8, 1))))
    return res


def _prep(src, dst):
    deg = np.bincount(dst, minlength=N_NODES).astype(np.int64)
    invdeg = (1.0 / np.maximum(deg, 1.0)).astype(np.float32)

    core = dst // NPC
    dloc = dst % NPC
    w_e = dloc // WIN
    par = (src & 1).astype(np.int64)
    drel = (dloc % WIN + WIN * par).astype(np.float32)
    sloc = src % NPC
    scor = src // NPC

    # ---- L1: single stream, gather from global x pair table ----
    rel = (dloc % WIN).astype(np.float32)
    bud1, kinds1, idx1_buf, drl1_buf, S1 = _stream_layout(
        core, w_e, par, rel, src >> 1)

    # ---- choose L2 piece boundaries (min estimated critical chain) ----
    l1_dma = S1 * NS_PER_SLOT            # µs
    best = None
    for bounds in PIECE_CANDS:
        offs = [0] + list(bounds) + [NPC]
        piece_slots = []
        tot = 0
        for p in range(len(offs) - 1):
            sel = (sloc >= offs[p]) & (sloc < offs[p + 1])
            c = np.zeros((M_CORES, NW), np.int64)
            np.add.at(c, (core[sel], w_e[sel]), 1)
            b = _round_up(c.max(axis=0), 128)
            if p == 0:
                b = np.maximum(b, 128)
            piece_slots.append(int(b.sum()))
            tot += int(b.sum())
        # chain: AGs serialize on the collective resource; last piece's
        # gathers wait for its AG
        t = 0.0
        for p in range(len(offs) - 1):
            ready = offs[p + 1] / NPC * l1_dma + 18.0
            nodes = offs[p + 1] - offs[p]
            ag = 15.0 + nodes * 8 * 128 / 40e3
            t = max(t, ready) + ag
        span = max(t + piece_slots[-1] * NS_PER_SLOT + 12.0,
                   (S1 + tot) * NS_PER_SLOT + 25.0)
        if best is None or span < best[0]:
            best = (span, offs)
    if os.environ.get("FORCE_OFFS"):
        best = (0, [0] + [int(x) for x in
                          os.environ["FORCE_OFFS"].split(",")] + [NPC])
    offs = best[1]
    npieces = len(offs) - 1
    assert all(o % SLAB == 0 for o in offs[1:-1])

    # ---- L2: one stream per piece, gather from h pair-piece tables ----
    bud2, kinds2, S2 = [], [], []
    idx2_bufs, drl2_bufs = [], []
    for p in range(npieces):
        np_p = (offs[p + 1] - offs[p]) // 2          # pairs/core in piece
        sel = (sloc >= offs[p]) & (sloc < offs[p + 1])
        gidx = scor[sel] * np_p + (sloc[sel] - offs[p]) // 2
        assert gidx.max() < 8 * np_p <= 32768
        b, kn, ib, db, S = _stream_layout(
            core[sel], w_e[sel], par[sel], rel[sel], gidx)
        bud2.append(b)
        kinds2.append(kn)
        idx2_bufs.append(ib)
        drl2_bufs.append(db)
        S2.append(S)

    st = dict(
        offs=offs, npieces=npieces,
        bud1=bud1, bud2=bud2,
        kinds1=kinds1, kinds2=kinds2,
        S1=S1, S2=S2,
        nch1=S1 // 128, nch2=[s // 128 for s in S2],
    )

    # per-core drt: [128, nch1 + sum(nch2)] f32 (L1 cols then L2 pieces)
    drt = []
    for c in range(M_CORES):
        cols = [drl1_buf[c].reshape(-1, 128).T]
        for p in range(npieces):
            cols.append(drl2_bufs[p][c].reshape(-1, 128).T)
        drt.append(np.ascontiguousarray(np.concatenate(cols, axis=1)))

    idx1 = _wrap_idx(idx1_buf)
    idx2 = []
    for c in range(M_CORES):
        blocks = [idx2_bufs[p][c] for p in range(npieces)]
        flat = np.concatenate(blocks)
        a = flat.astype(np.int16).reshape(-1, 16).T
        idx2.append(np.ascontiguousarray(np.tile(a, (8, 1))))

    pc = dict(drt=drt, idx1=idx1, idx2=idx2, invdeg=invdeg)
    return st, pc


def _build_bass(st, debug=False):
    import concourse.bass as bass
    import concourse.mybir as mybir
    import concourse.tile as tile
    from concourse.ap import AP
    from concourse import bacc, library_config

    f32 = mybir.dt.float32
    f32r = mybir.dt.float32r
    fp16 = mybir.dt.float16
    i16 = mybir.dt.int16
    AF = mybir.ActivationFunctionType
    OP = mybir.AluOpType

    offs = st["offs"]
    npieces = st["npieces"]
    nch1 = st["nch1"]
    nch2 = st["nch2"]
    NCH = nch1 + sum(nch2)
    np_p = [(offs[p + 1] - offs[p]) // 2 for p in range(npieces)]

    nc = bacc.Bacc(None, target_bir_lowering=False)

    xpair_d = nc.dram_tensor("xpair", [N_NODES // 2, 128], fp16,
                             kind="ExternalInput")
    xT_d = nc.dram_tensor("xT", [IN_F, NPC], f32r, kind="ExternalInput")
    w1c_d = nc.dram_tensor("w1c", [2 * IN_F, HID], f32r, kind="ExternalInput")
    w2c_d = nc.dram_tensor("w2c", [2 * HID, OUT_C], f32r, kind="ExternalInput")
    b1_d = nc.dram_tensor("b1c", [HID, 1], f32, kind="ExternalInput")
    b2_d = nc.dram_tensor("b2c", [OUT_C, 1], f32, kind="ExternalInput")
    iota_d = nc.dram_tensor("iota", [128, 128], fp16, kind="ExternalInput")
    ident_d = nc.dram_tensor("ident", [IN_F, IN_F], f32, kind="ExternalInput")
    invd_d = nc.dram_tensor("invd", [64, NPC], f32, kind="ExternalInput")
    drel_d = nc.dram_tensor("dstrel", [128, NCH], f32, kind="ExternalInput")
    idx1_d = nc.dram_tensor("idx1", [128, st["S1"] // 16], i16,
                            kind="ExternalInput")
    idx2_d = nc.dram_tensor("idx2", [128, sum(st["S2"]) // 16], i16,
                            kind="ExternalInput")
    out_d = nc.dram_tensor("out", [OUT_C, NPC], f32, kind="ExternalOutput")
    scratch_d = nc.dram_tensor("scratch", [1, 64], fp16)
    if debug:
        dbg_z1_d = nc.dram_tensor("dbg_z1", [128, NPC], f32,
                                  kind="ExternalOutput")
        dbg_z2_d = nc.dram_tensor("dbg_z2", [128, NPC], f32,
                                  kind="ExternalOutput")

    h_shard = [nc.dram_tensor(f"h_shard_{p}", [2 * np_p[p], HID], fp16)
               for p in range(npieces)]
    h_table = [nc.dram_tensor(f"h_table_{p}", [2 * 8 * np_p[p], HID], fp16,
                              addr_space="Shared")
               for p in range(npieces)]

    with tile.TileContext(nc) as tc:
        nc.gpsimd.load_library(library_config.mlp)
        with (
            tc.tile_pool(name="const", bufs=1) as cpool,
            tc.tile_pool(name="gath", bufs=int(os.environ.get("GBUFS", 3))) as gpool,
            tc.tile_pool(name="oh", bufs=int(os.environ.get("GBUFS", 3))) as ohpool,
            tc.tile_pool(name="hsb", bufs=12) as hpool,
            tc.tile_pool(name="osl", bufs=3) as opool,
            tc.tile_pool(name="agg", bufs=6, space="PSUM") as apool,
            tc.tile_pool(name="msc", bufs=1, space="PSUM") as mpool,
            tc.tile_pool(name="prj", bufs=1, space="PSUM") as jpool,
        ):
            # ---- persistent SBUF ----
            z1 = cpool.tile([2 * IN_F, NPC], f32r, tag="z1")
            z2 = cpool.tile([2 * HID, NPC], f32r, tag="z2")
            w1t = cpool.tile([2 * IN_F, HID], f32r, tag="w1t")
            w2t = cpool.tile([2 * HID, OUT_C], f32r, tag="w2t")
            b1t = cpool.tile([HID, 1], f32, tag="b1t")
            b2t = cpool.tile([OUT_C, 1], f32, tag="b2t")
            iot = cpool.tile([128, 128], fp16, tag="iot")
            idt = cpool.tile([IN_F, IN_F], f32, tag="idt")
            ivt = cpool.tile([128, NPC], f32, tag="ivt")
            drt = cpool.tile([128, NCH], f32, tag="drt")
            ix1 = cpool.tile([128, st["S1"] // 16], i16, tag="ix1")
            ix2 = cpool.tile([128, sum(st["S2"]) // 16], i16, tag="ix2")

            # load order matters: the DMA engines are FIFO, so the first
            # gather call queues behind whatever consts precede it.  Load
            # only the L1-stream-critical prefix first (idx/dstrel split
            # so subtile deps release the first gather early); defer the
            # L2-only loads behind the first h write (SP queue blocks on
            # it, staggering them off the head of the DMA FIFO).
            # staged input loads: the DMA engines are FIFO, so anything
            # loaded before the gathers a piece-1 AllGather depends on
            # delays the whole collective chain.  stage0 = minimal stream
            # prefix; stage2 (after call 1, held back by a blocker DMA
            # reading call-1's onehot) = what layer-1 slabs 0..7 need;
            # stage3 (after piece-1's h is written) = everything else.
            IX1H = min(2560 // 16 * 16, st["S1"] // 16)
            DRTH = min(320, NCH)
            XTH = min(4096, NPC)
            nc.sync.dma_start(ix1[:, 0:IX1H], idx1_d[:, 0:IX1H])
            nc.sync.dma_start(iot[:], iota_d[:])
            nc.sync.dma_start(drt[:, 0:DRTH], drel_d[:, 0:DRTH])

            last_oh = [None]
            stage2 = [False]

            def load_stage2():
                nc.sync.dma_start(scratch_d[0:1, :],
                                  last_oh[0][0:1, 0, 0:64])
                nc.sync.dma_start(w1t[:], w1c_d[:])
                nc.sync.dma_start(b1t[:], b1_d[:])
                nc.sync.dma_start(z1[0:IN_F, 0:XTH], xT_d[:, 0:XTH])
                nc.sync.dma_start(ivt[64:128, 0:XTH], invd_d[:, 0:XTH])
                nc.sync.dma_start(idt[:], ident_d[:])
                stage2[0] = True

            deferred = [False]

            def load_deferred():
                nc.sync.dma_start(ix1[:, IX1H:], idx1_d[:, IX1H:])
                nc.sync.dma_start(drt[:, DRTH:], drel_d[:, DRTH:])
                nc.sync.dma_start(z1[0:IN_F, XTH:], xT_d[:, XTH:])
                nc.sync.dma_start(ivt[64:128, XTH:], invd_d[:, XTH:])
                nc.sync.dma_start(ix2[:], idx2_d[:])
                nc.sync.dma_start(w2t[:], w2c_d[:])
                nc.sync.dma_start(b2t[:], b2_d[:])
                deferred[0] = True

            # misc psum bank: [:, 0:64] = windows 96/97, [:, 64+64j] = L1
            # transpose slots
            misc = mpool.tile([128, SLAB], f32, tag="misc")

            # ---------------- shared machinery ----------------
            def win_slice(wtiles, w, wn):
                t = w // 2
                if t == 48:
                    tl = misc
                    col = 0
                else:
                    tl = wtiles[t // 8]
                    col = (t % 8) * 64
                ph = (w % 2) * 64
                return tl[ph: ph + 64, col: col + wn]

            def emit_stream(chunks_per_win, kinds, kbase, ixt, ix_off,
                            table_ap, wtiles, alloc_group, on_call_end,
                            depth=None, pipe=None):
                """Issue gather/onehot/matmul stream, software-pipelined:
                gathers + onehots run PIPE calls ahead of the matmuls, so
                a close-op stall at the DVE queue head never starves the
                PE.  Each window's psum accumulation group is contiguous
                WITHIN this stream.  Parity-pure chunks ('E'/'O') take one
                matmul with the matching lhsT half; mixed chunks use the
                folded dstrel + two."""
                chlist = []
                for w in range(NW):
                    for j in range(chunks_per_win[w]):
                        chlist.append((w, j))
                calls = []
                k = 0
                while k < len(chlist):
                    nb = min(GB // 128, len(chlist) - k)
                    calls.append((k, nb))
                    k += nb
                PIPE = pipe if pipe is not None \
                    else int(os.environ.get('PIPE', 1))
                DEP = depth if depth is not None \
                    else int(os.environ.get("GBUFS", 3))
                tiles = {}

                def front(ci):
                    k, nb = calls[ci]
                    b0 = k * 128
                    g = gpool.tile([128, GB // 128, 128], fp16,
                                   tag=f"g{DEP}", bufs=DEP, name="g")
                    nc.gpsimd.dma_gather(
                        out_ap=g[:, 0:nb, :],
                        in_ap=table_ap,
                        idxs_ap=ixt[:, ix_off + b0 // 16:
                                    ix_off + b0 // 16 + nb * 8],
                        num_idxs=nb * 128,
                        num_idxs_reg=nb * 128,
                        elem_size=128,
                        single_packet=False,
                    )
                    oh = ohpool.tile([128, GB // 128, 128], fp16,
                                     tag=f"oh{DEP}", bufs=DEP, name="oh")
                    last_oh[0] = oh
                    for col in range(nb):
                        kind = kinds[k + col]
                        ohw = 128 if kind == "M" else 64
                        # pure onehot (iota == dstrel), fp16, 4x DVE mode
                        nc.vector.tensor_scalar(
                            out=oh[:, col, 0:ohw],
                            in0=iot[:, 0:ohw],
                            scalar1=drt[:, kbase + k + col:
                                        kbase + k + col + 1],
                            scalar2=None,
                            op0=OP.is_equal,
                        )
                    tiles[ci] = (g, oh)

                def back(ci):
                    k, nb = calls[ci]
                    g, oh = tiles.pop(ci)
                    for col in range(nb):
                        w, j = chlist[k + col]
                        kind = kinds[k + col]
                        wn = min(WIN, NPC - w * WIN)
                        if (w // 2) != 48 and (w // 16) not in wtiles:
                            wtiles[w // 16] = alloc_group()
                        ps = win_slice(wtiles, w, wn)
                        first = (j == 0)
                        last = (j == chunks_per_win[w] - 1)
                        if kind == "M":
                            nc.tensor.matmul(
                                ps, g[:, col, 0:64], oh[:, col, 0:wn],
                                start=first, stop=False)
                            nc.tensor.matmul(
                                ps, g[:, col, 64:128],
                                oh[:, col, 64:64 + wn],
                                start=False, stop=last)
                        else:
                            half = slice(0, 64) if kind == "E" \
                                else slice(64, 128)
                            nc.tensor.matmul(
                                ps, g[:, col, half], oh[:, col, 0:wn],
                                start=first, stop=last)
                    on_call_end(ci + 1, chlist[k + nb - 1][0])

                for ci in range(len(calls)):
                    front(ci)
                    if ci >= PIPE:
                        back(ci - PIPE)
                for ci in range(max(0, len(calls) - PIPE), len(calls)):
                    back(ci)

            def close_groups(layer, z, wtiles, upto_g, state, add=False,
                             run_slabs=True):
                """Close whole 16-window psum groups <= upto_g: one strided
                ACT copy (or DVE add) per (bank tile, partition half)
                moves 8 windows at once; then run slab completions."""
                ng = 7  # groups 0..5 full tiles, group 6 = windows 96/97
                while state["g"] <= min(upto_g, ng - 1):
                    g = state["g"]
                    if g < 6:
                        tl = wtiles[g]
                        for ph in (0, 64):
                            # even (ph=0) / odd (ph=64) windows of group
                            zb = z[HID:, (16 * g + ph // 64) * WIN:
                                   (16 * g + ph // 64) * WIN + WIN]
                            zsl = AP(zb.tensor, zb.offset,
                                     [zb.ap[0], [128, 8], [1, WIN]])
                            pb = tl[ph: ph + 64, 0:SLAB]
                            psl = AP(pb.tensor, pb.offset,
                                     [pb.ap[0], [WIN, 8], [1, WIN]])
                            if not add:
                                nc.scalar.copy(zsl, psl)
                            else:
                                nc.vector.scalar_tensor_tensor(
                                    out=zsl, in0=psl, scalar=1.0, in1=zsl,
                                    op0=OP.mult, op1=OP.add)
                    else:
                        for w in (96, 97):
                            wn = min(WIN, NPC - w * WIN)
                            zsl = z[HID:, w * WIN: w * WIN + wn]
                            ps = win_slice(wtiles, w, wn)
                            if not add:
                                nc.scalar.copy(zsl, ps)
                            else:
                                nc.vector.scalar_tensor_tensor(
                                    out=zsl, in0=ps, scalar=1.0, in1=zsl,
                                    op0=OP.mult, op1=OP.add)
                    state["g"] += 1
                    if run_slabs:
                        for s in (2 * g, 2 * g + 1):
                            if s * SLAB < NPC:
                                finish_slab(layer, z, s)

            def finish_slab(layer, z, s):
                a, b = s * SLAB, min((s + 1) * SLAB, NPC)
                cols = b - a
                # mean division (invdeg folded out of the onehots)
                nc.vector.scalar_tensor_tensor(
                    out=z[HID:, a:b], in0=z[HID:, a:b], scalar=1.0,
                    in1=ivt[64:128, a:b].bitcast(f32r),
                    op0=OP.mult, op1=OP.mult)
                if layer == 1:
                    p1 = jpool.tile([HID, SLAB], f32, tag="prj",
                                    name="prj")
                    nc.tensor.matmul(p1[:, :cols], w1t[:], z[:, a:b],
                                     start=True, stop=True)
                    nc.scalar.activation(z2[0:HID, a:b], p1[:, :cols],
                                         AF.Relu, bias=b1t[:, 0:1])
                    for j4 in range(-(-cols // 128)):
                        ca = a + j4 * 128
                        cb = min(ca + 128, b)
                        cc = cb - ca
                        tp = misc[:, 64 + (j4 % 4) * 64:
                                  128 + (j4 % 4) * 64]
                        nc.tensor.transpose(tp[:cc, :],
                                            z2[0:HID, ca:cb].bitcast(f32),
                                            idt[:])
                        hs = hpool.tile([128, HID], fp16, tag="hs",
                                        name="hs")
                        nc.scalar.copy(hs[:cc, :], tp[:cc, :])
                        # piece containing this chunk
                        p = next(i for i in range(npieces)
                                 if offs[i] <= ca < offs[i + 1])
                        nc.sync.dma_start(
                            h_shard[p][ca - offs[p]: cb - offs[p], :],
                            hs[:cc, :])
                    if not deferred[0] and b >= offs[1]:
                        load_deferred()
                    # queue AllGathers for completed h pieces (all but the
                    # last, which is deferred into the L2 piece-0 stream)
                    for p in range(npieces - 1):
                        if not ag_emitted[p] and b >= offs[p + 1]:
                            ag_pending.append(
                                (p, ag_ready[-1] if ag_ready else 0))
                            ag_emitted[p] = True
                else:
                    p2 = jpool.tile([HID, SLAB], f32, tag="prj",
                                    name="prj")[0:OUT_C, :]
                    nc.tensor.matmul(p2[:, :cols], w2t[:], z[:, a:b],
                                     start=True, stop=True)
                    osl = opool.tile([OUT_C, SLAB], f32, tag="osl",
                                     name="osl")
                    nc.scalar.activation(osl[:, :cols], p2[:, :cols],
                                         AF.Identity, bias=b2t[:, 0:1])
                    nc.sync.dma_start(out_d[:, a:b], osl[:, :cols])

            def emit_ag(p):
                nc.gpsimd.collective_compute(
                    "AllGather",
                    mybir.AluOpType.bypass,
                    replica_groups=[list(range(M_CORES))],
                    ins=[h_shard[p][:]],
                    outs=[h_table[p][:]],
                )

            # ================= layer 1 =================
            wt1 = {}
            st1 = {"g": 0}
            ag_emitted = [False] * npieces
            ag_pending = []      # (piece, ready_at_call)
            ag_ready = []

            def alloc_agg():
                return apool.tile([128, SLAB], f32, tag="agg", name="agg")

            def l1_call_end(ncall, last_w):
                if ncall == 1 and not stage2[0]:
                    load_stage2()
                close_groups(1, z1, wt1, last_w // 16 - 1, st1)
                # emit pending AllGathers a few calls after their h piece
                # completed, so their sem waits never head-block the Pool
                # queue ahead of gather dispatches
                while ag_pending and ncall >= ag_pending[0][1] + AG_DELAY:
                    emit_ag(ag_pending.pop(0)[0])
                ag_ready.append(ncall)

            emit_stream(st["bud1"], st["kinds1"], 0, ix1, 0, xpair_d[:],
                        
            close_groups(1, z1, wt1, 6, st1)
            while ag_pending:
                emit_ag(ag_pending.pop(0)[0])

            # ================= layer 2 =================
            kbase = nch1
            ix_off = 0
            for p in range(npieces):
                wt2 = {}
                st2 = {"g": 0}
                lastp = (p == npieces - 1)

                def call_end(ncall, last_w, _p=p, _wt=wt2, _st=st2,
                             _lp=lastp):
                    if (_p == 0 and ncall == AG_AFTER_P1_CALLS
                            and not ag_emitted[npieces - 1]):
                        # last h piece is complete by now; emitting here
                        # keeps its sem wait from blocking the Pool SEQ
                        # ahead of the piece-0 gather dispatches
                        emit_ag(npieces - 1)
                        ag_emitted[npieces - 1] = True
                    close_groups(2, z2, _wt, last_w // 16 - 1, _st,
                                 add=(_p > 0), run_slabs=_lp)

                if p == npieces - 1 and not ag_emitted[p]:
                    emit_ag(p)          # safety: piece-0 stream was short
                    ag_emitted[p] = True
                base = h_table[p][:]
                tab = AP(base.tensor, 0, [[128, 8 * np_p[p]], [1, 128]])
                emit_stream(st["bud2"][p], st["kinds2"][p], kbase, ix2,
                            ix_off, tab, wt2, alloc_agg, call_end,
                            depth=6, pipe=1)
                close_groups(2, z2, wt2, 6, st2, add=(p > 0),
                             run_slabs=lastp)
                kbase += st["nch2"][p]
                ix_off += st["S2"][p] // 16
            if debug:
                nc.sync.dma_start(dbg_z1_d[:], z1[:].bitcast(f32))
                nc.sync.dma_start(dbg_z2_d[:], z2[:].bitcast(f32))

    nc.compile()
    return nc


def _make_in_maps(features, W_self1, W_neigh1, b1, W_self2, W_neigh2, b2,
                  st, pc):
    feat = np.ascontiguousarray(features, dtype=np.float32)
    xpair = feat.astype(np.float16).reshape(N_NODES // 2, 128)
    w1c = np.vstack([W_self1, W_neigh1]).astype(np.float32)
    w2c = np.vstack([W_self2, W_neigh2]).astype(np.float32)
    b1c = np.asarray(b1, np.float32).reshape(-1, 1)
    b2c = np.asarray(b2, np.float32).reshape(-1, 1)
    iota = np.tile(np.arange(128, dtype=np.float16), (128, 1))
    ident = np.eye(IN_F, dtype=np.float32)
    NW_ = NW
    in_maps = []
    for c in range(M_CORES):
        sl = slice(c * NPC, (c + 1) * NPC)
        ivd = np.ascontiguousarray(
            np.tile(pc["invdeg"][sl], (64, 1)))
        in_maps.append({
            "xpair": xpair,
            "xT": np.ascontiguousarray(feat[sl].T),
            "w1c": w1c, "w2c": w2c, "b1c": b1c, "b2c": b2c,
            "iota": iota, "ident": ident,
            "invd": ivd,
            "dstrel": pc["drt"][c],
            "idx1": pc["idx1"][c],
            "idx2": pc["idx2"][c],
        })
    return in_maps


_TRACE_RESULT = {}


def kernel(features, W_self1, W_neigh1, b1, W_self2, W_neigh2, b2, src, dst,
           _trace=False):
    from concourse.bass_utils import run_bass_kernel_spmd

    features = np.asarray(features, np.float32)
    src = np.asarray(src, np.int64)
    dst = np.asarray(dst, np.int64)

    # relabel nodes to flatten per-window in-degree sums (less padding)
    deg = np.bincount(dst, minlength=N_NODES)
    perm = _balance_permutation(deg)
    inv = np.empty(N_NODES, np.int64)
    inv[perm] = np.arange(N_NODES)

    st, pc = _prep(inv[src], inv[dst])
    nc = _build_bass(st)
    in_maps = _make_in_maps(features[perm], W_self1, W_neigh1, b1,
                            W_self2, W_neigh2, b2, st, pc)
    est_ns = None
    if _trace:
        try:
            from concourse.timeline_sim import TimelineSim
            ts = TimelineSim(nc, no_exec=True)
            ts.simulate()
            est_ns = int(ts.time)
        except Exception:
            import traceback
            traceback.print_exc()
    res = run_bass_kernel_spmd(nc, in_maps, core_ids=list(range(M_CORES)),
                               trace=False)
    exec_ns = res.exec_time_ns if res.exec_time_ns is not None else est_ns
    _TRACE_RESULT.clear()
    _TRACE_RESULT.update(dict(exec_time_ns=exec_ns,
                              trace=res.instructions_and_trace))
    out = np.concatenate([r["out"].T for r in res.results], axis=0)
    res_full = np.empty_like(out)
    res_full[perm] = out           # un-permute rows to original node ids
    return res_full.astype(np.float32)


# revision 11
# speedup vs baseline: 1.0458x; 1.0078x over previous
"""Trainium2 Bass kernel for a 2-layer mean-aggregation GraphSAGE GNN.

Strategy (8 NeuronCores, SPMD single program):
  - Destination nodes sharded contiguously across cores (6250/core), with
    a host-side node relabeling that (a) balances total in-degree across
    cores and (b) packs each 64-dst PSUM window to <= 1024 incoming edges
    (greedy + swap repair), so the max-over-cores window budgets hit the
    128-slot quantization exactly (L1 stream = 100096 slots vs 112512
    unbalanced).
  - Features / hidden activations stored as FP16 PAIR tables ([n/2, 128]
    rows = two consecutive node rows = 256B, the dma_gather granularity),
    so gather indices are pair ids < 32768 (int16) and no A/B table split
    is needed.  Parity (which pair half) is handled per chunk: where the
    even/odd split costs no padding the slots are parity-sorted and each
    chunk takes ONE fp16 matmul with the matching lhsT half; otherwise
    parity folds into the dstrel (+64) compared against a width-128 iota
    with two matmuls per chunk.
  - One-hots are built per chunk with tensor_scalar(is_equal) in fp16 —
    all-SBUF 2-byte operands hit the DVE 4x perf mode (~93ns/chunk).
    Mean division is applied per 512-col slab after the psum closes.
  - The gather/onehot/matmul stream is software-pipelined (front-end 2
    calls ahead), so close-op stalls at the DVE queue head never starve
    the PE; input loads are staged in three waves so the DMA-engine FIFO
    head stays clear for the gathers feeding the first AllGather.
  - PSUM: two windows per [128, 512] bank tile ((w%2)*64 partition half,
    ((w//2)%8)*64 column), rolling groups of 16 windows; each stream
    closes a group with ONE strided ACT copy (or DVE add for later
    layer-2 pieces) per partition half — accumulation groups stay
    contiguous per stream (required: split start/stop groups corrupt).
  - h exchange: 3 AllGathers over fp16 pair-row pieces of the local node
    range (boundaries chosen per input from candidates, slab-aligned),
    emitted a few calls after their h piece completes so their sem waits
    never head-block the Pool queue; layer-2 slot streams are split by
    src piece so gathers fire as each AllGather lands.
  - Projections via float32r matmuls over 512-col slabs (1 cycle/row).
  - Final [32, 6250] per-core output is transposed/concatenated and
    un-permuted on host.
"""

import os
import sys

import numpy as np

for _p in ("/opt/trn_rl_repo", "/root/.axon_site/_ro/trn_rl_repo"):
    if os.path.isdir(_p) and _p not in sys.path:
        sys.path.append(_p)

# ---- problem constants (hardcoded per harness contract) ----
N_NODES = 50000
N_EDGES = 800000
IN_F = 64
HID = 64
OUT_C = 32
M_CORES = 8
NPC = N_NODES // M_CORES     # 6250
WIN = 64
NW = -(-NPC // WIN)          # 98
GB = 2048                    # slots per dma_gather call
SLAB = 512                   # projection slab (psum bank cols)
# piece boundaries must be SLAB multiples (h DMAs never straddle pieces)
PIECE_CANDS = [(2048, 4096), (1536, 4096), (2048, 4608), (2560, 4608),
               (2048, 3584), (3072,), (3584,), (2560,)]
AG_AFTER_P1_CALLS = 8        # issue last AllGather after this many L2p1 calls
AG_DELAY = int(os.environ.get('AG_DELAY', 3))   # calls between h-piece completion and its AG
NS_PER_SLOT = 1.4225e-3      # µs, gather DMA cost per slot (cost model)


def _round_up(x, k):
    return (x + k - 1) // k * k


def _balance_permutation(deg):
    """Relabel nodes so per-core and per-window in-degree sums are flat,
    minimizing the round-128 padding of the max-over-cores window budgets.
    Returns perm (new position -> original node id)."""
    import heapq

    order = np.argsort(-deg, kind="stable")
    # 1) balance total in-degree across cores (capacity NPC each)
    heap = [(0, c) for c in range(M_CORES)]
    heapq.heapify(heap)
    cap = [NPC] * M_CORES
    core_nodes = [[] for _ in range(M_CORES)]
    for v in order:
        picked = []
        while True:
            s, c = heapq.heappop(heap)
            if cap[c] > 0:
                break
            picked.append((s, c))
        core_nodes[c].append(v)
        cap[c] -= 1
        heapq.heappush(heap, (s + int(deg[v]), c))
        for it in picked:
            heapq.heappush(heap, it)

    # 2) within each core, pack windows toward exact 1024-edge targets
    perm = np.empty(N_NODES, np.int64)
    lastcap = NPC - (NW - 1) * WIN
    for c in range(M_CORES):
        nodes = core_nodes[c]                     # desc by degree
        tot = int(deg[nodes].sum())
        t_last = max(tot - (NW - 1) * 8 * 128, 8 * int(deg[nodes[0]]))
        # heap of (-slack, w); assign heaviest node to max-slack window
        caps = [WIN] * (NW - 1) + [lastcap]
        tgts = [8 * 128] * (NW - 1) + [t_last]
        sums = [0] * NW
        cnts = [0] * NW
        heap2 = [(-tgts[w], w) for w in range(NW)]
        heapq.heapify(heap2)
        wassign = [[] for _ in range(NW)]
        for v in nodes:
            picked = []
            while True:
                negslack, w = heapq.heappop(heap2)
                if cnts[w] < caps[w]:
                    break
                picked.append((negslack, w))
            wassign[w].append(v)
            cnts[w] += 1
            sums[w] += int(deg[v])
            heapq.heappush(heap2, (sums[w] - tgts[w], w))
            for it in picked:
                heapq.heappush(heap2, it)
        # 3) repair pass: swap nodes between windows so no full window
        # exceeds the 1024-edge (8-chunk) budget bin; the short last
        # window may absorb up to its 768 bin.
        BIN = 8 * 128
        from collections import defaultdict
        wdeg = [defaultdict(list) for _ in range(NW)]   # deg -> [nodes]
        for w in range(NW):
            for v in wassign[w]:
                wdeg[w][int(deg[v])].append(v)

        def room(u):
            cap_u = BIN if u < NW - 1 else 768
            return cap_u - sums[u]

        for w in range(NW - 1):
            guard = 0
            while sums[w] > BIN and guard < 200:
                guard += 1
                need = sums[w] - BIN
                done = False
                for da in sorted(wdeg[w], reverse=True):
                    if not wdeg[w][da]:
                        continue
                    for k in range(min(need, da - 1), 0, -1):
                        db = da - k
                        for u in range(NW):
                            if (u == w or room(u) < k
                                    or not wdeg[u].get(db)):
                                continue
                            a = wdeg[w][da].pop()
                            b = wdeg[u][db].pop()
                            wdeg[w][db].append(b)
                            wdeg[u][da].append(a)
                            ia = wassign[w].index(a)
                            ib = wassign[u].index(b)
                            wassign[w][ia] = b
                            wassign[u][ib] = a
                            sums[w] -= k
                            sums[u] += k
                            done = True
                            break
                        if done:
                            break
                    if done:
                        break
                if not done:
                    break

        # 4) parity balance: within each window, order nodes so the
        # even-position half and odd-position half carry (<= bud/2) each
        # -> layer-1 chunks are parity-pure with zero extra padding
        for w in range(NW):
            nodes = sorted(wassign[w], key=lambda v: -deg[v])
            hcap = _round_up(max(sums[w], 1), 128) // 2
            bins = [[], []]
            bsum = [0, 0]
            bcap = [(len(nodes) + 1) // 2, len(nodes) // 2]
            for v in nodes:
                t = 0 if (bsum[0] <= bsum[1] and len(bins[0]) < bcap[0]) \
                    else (1 if len(bins[1]) < bcap[1] else 0)
                bins[t].append(v)
                bsum[t] += int(deg[v])
            for _ in range(64):
                hi = 0 if bsum[0] >= bsum[1] else 1
                excess = bsum[hi] - hcap
                if excess <= 0:
                    break
                done = False
                for a in sorted(bins[hi], key=lambda v: -deg[v]):
                    for b in sorted(bins[1 - hi], key=lambda v: deg[v]):
                        k = int(deg[a]) - int(deg[b])
                        if 1 <= k <= excess and bsum[1 - hi] + k <= hcap:
                            bins[hi].remove(a)
                            bins[1 - hi].remove(b)
                            bins[hi].append(b)
                            bins[1 - hi].append(a)
                            bsum[hi] -= k
                            bsum[1 - hi] += k
                            done = True
                            break
                    if done:
                        break
                if not done:
                    break
            merged = []
            for i in range(len(nodes)):
                merged.append(bins[i % 2][i // 2])
            wassign[w] = merged

        pos = c * NPC
        for w in range(NW):
            for v in wassign[w]:
                perm[pos] = v
                pos += 1
        assert pos == (c + 1) * NPC
    return perm


def _stream_layout(core, w, par, rel, gidx):
    """Parity-sorted padded slot streams for one gather stream.

    Within each window segment: even-parity slots first, padded to the
    shared boundary E_w = max-over-cores even count; odd slots after.
    Chunks below/above the boundary are parity-pure and take ONE matmul
    with the matching lhsT half; the (single) straddling chunk is mixed
    and uses the parity-folded dstrel + two matmuls.

    Returns (budgets_chunks[NW], kinds per chunk 'E'/'O'/'M',
             idx [M, S], drl [M, S], S).
    """
    ce = np.zeros((M_CORES, NW), np.int64)
    co = np.zeros((M_CORES, NW), np.int64)
    np.add.at(ce, (core[par == 0], w[par == 0]), 1)
    np.add.at(co, (core[par == 1], w[par == 1]), 1)
    E = ce.max(axis=0)
    O = co.max(axis=0)
    J = (ce + co).max(axis=0)
    bud = np.maximum(_round_up(J, 128), 128)
    # sorted mode only where the parity split costs no extra padding;
    # otherwise fold parity into dstrel (all chunks mixed)
    smode = _round_up(E + O, 128) <= bud
    # even/odd boundary: chunk-aligned when the odds still fit
    B = np.where(_round_up(E, 128) + O <= bud, _round_up(E, 128), E)
    B = np.where(smode, B, 0)
    seg_off = np.concatenate([[0], np.cumsum(bud)])
    S = int(seg_off[-1])

    # chunk kinds (shared across cores)
    kinds = []
    for wi in range(NW):
        nch = int(bud[wi]) // 128
        for j in range(nch):
            lo, hi = j * 128, (j + 1) * 128
            if not smode[wi]:
                kinds.append("M")
            elif hi <= B[wi]:
                kinds.append("E")
            elif lo >= B[wi]:
                kinds.append("O")
            else:
                kinds.append("M")

    # slot positions: sorted windows rank evens from seg start, odds
    # from B; folded windows rank jointly
    pcls = np.where(smode[w], par, 0)
    key = (core * NW + w) * 2 + pcls
    order = np.argsort(key, kind="stable")
    ks = key[order]
    grp_start = np.searchsorted(ks, np.arange(M_CORES * NW * 2), side="left")
    ranks = np.arange(len(ks)) - grp_start[ks]
    wo = w[order]
    po = par[order]
    pos = seg_off[wo] + np.where(smode[wo] & (po == 1), B[wo], 0) + ranks

    # dstrel: fold +64*parity for folded windows and mixed-chunk odds
    ch_in_seg = (pos - seg_off[wo]) // 128
    mixed = (ch_in_seg * 128 < B[wo]) & ((ch_in_seg + 1) * 128 > B[wo])
    fold = (~smode[wo]) | (mixed & (po == 1))
    drl_o = np.where(fold & (po == 1), rel[order] + WIN,
                     rel[order]).astype(np.float32)

    idx_buf = np.zeros((M_CORES, S), np.int64)
    drl_buf = np.full((M_CORES, S), -1.0, np.float32)
    idx_buf[core[order], pos] = gidx[order]
    drl_buf[core[order], pos] = drl_o
    return [int(b) // 128 for b in bud], kinds, idx_buf, drl_buf, S


def _wrap_idx(streams):
    """[M, S] int -> per-core [128, S/16] int16 gather-index layout."""
    res = []
    for c in range(M_CORES):
        a = streams[c].astype(np.int16).reshape(-1, 16).T
        res.append(np.ascontiguousarray(np.tile(a, (8, 1))))
    return res


def _prep(src, dst):
    deg = np.bincount(dst, minlength=N_NODES).astype(np.int64)
    invdeg = (1.0 / np.maximum(deg, 1.0)).astype(np.float32)

    core = dst // NPC
    dloc = dst % NPC
    w_e = dloc // WIN
    par = (src & 1).astype(np.int64)
    drel = (dloc % WIN + WIN * par).astype(np.float32)
    sloc = src % NPC
    scor = src // NPC

    # ---- L1: single stream, gather from global x pair table ----
    rel = (dloc % WIN).astype(np.float32)
    bud1, kinds1, idx1_buf, drl1_buf, S1 = _stream_layout(
        core, w_e, par, rel, src >> 1)

    # ---- choose L2 piece boundaries (min estimated critical chain) ----
    l1_dma = S1 * NS_PER_SLOT            # µs
    best = None
    for bounds in PIECE_CANDS:
        offs = [0] + list(bounds) + [NPC]
        piece_slots = []
        tot = 0
        for p in range(len(offs) - 1):
            sel = (sloc >= offs[p]) & (sloc < offs[p + 1])
            c = np.zeros((M_CORES, NW), np.int64)
            np.add.at(c, (core[sel], w_e[sel]), 1)
            b = _round_up(c.max(axis=0), 128)
            if p == 0:
                b = np.maximum(b, 128)
            piece_slots.append(int(b.sum()))
            tot += int(b.sum())
        # chain: AGs serialize on the collective resource; last piece's
        # gathers wait for its AG
        t = 0.0
        for p in range(len(offs) - 1):
            ready = offs[p + 1] / NPC * l1_dma + 18.0
            nodes = offs[p + 1] - offs[p]
            ag = 15.0 + nodes * 8 * 128 / 40e3
            t = max(t, ready) + ag
        span = max(t + piece_slots[-1] * NS_PER_SLOT + 12.0,
                   (S1 + tot) * NS_PER_SLOT + 25.0)
        if best is None or span < best[0]:
            best = (span, offs)
    if os.environ.get("FORCE_OFFS"):
        best = (0, [0] + [int(x) for x in
                          os.environ["FORCE_OFFS"].split(",")] + [NPC])
    offs = best[1]
    npieces = len(offs) - 1
    assert all(o % SLAB == 0 for o in offs[1:-1])

    # ---- L2: one stream per piece, gather from h pair-piece tables ----
    bud2, kinds2, S2 = [], [], []
    idx2_bufs, drl2_bufs = [], []
    for p in range(npieces):
        np_p = (offs[p + 1] - offs[p]) // 2          # pairs/core in piece
        sel = (sloc >= offs[p]) & (sloc < offs[p + 1])
        gidx = scor[sel] * np_p + (sloc[sel] - offs[p]) // 2
        assert gidx.max() < 8 * np_p <= 32768
        b, kn, ib, db, S = _stream_layout(
            core[sel], w_e[sel], par[sel], rel[sel], gidx)
        bud2.append(b)
        kinds2.append(kn)
        idx2_bufs.append(ib)
        drl2_bufs.append(db)
        S2.append(S)

    st = dict(
        offs=offs, npieces=npieces,
        bud1=bud1, bud2=bud2,
        kinds1=kinds1, kinds2=kinds2,
        S1=S1, S2=S2,
        nch1=S1 // 128, nch2=[s // 128 for s in S2],
    )

    # per-core drt: [128, nch1 + sum(nch2)] f32 (L1 cols then L2 pieces)
    drt = []
    for c in range(M_CORES):
        cols = [drl1_buf[c].reshape(-1, 128).T]
        for p in range(npieces):
            cols.append(drl2_bufs[p][c].reshape(-1, 128).T)
        drt.append(np.ascontiguousarray(np.concatenate(cols, axis=1)))

    idx1 = _wrap_idx(idx1_buf)
    idx2 = []
    for c in range(M_CORES):
        blocks = [idx2_bufs[p][c] for p in range(npieces)]
        flat = np.concatenate(blocks)
        a = flat.astype(np.int16).reshape(-1, 16).T
        idx2.append(np.ascontiguousarray(np.tile(a, (8, 1))))

    pc = dict(drt=drt, idx1=idx1, idx2=idx2, invdeg=invdeg)
    return st, pc


def _build_bass(st, debug=False):
    import concourse.bass as bass
    import concourse.mybir as mybir
    import concourse.tile as tile
    from concourse.ap import AP
    from concourse import bacc, library_config

    f32 = mybir.dt.float32
    f32r = mybir.dt.float32r
    fp16 = mybir.dt.float16
    i16 = mybir.dt.int16
    AF = mybir.ActivationFunctionType
    OP = mybir.AluOpType

    offs = st["offs"]
    npieces = st["npieces"]
    nch1 = st["nch1"]
    nch2 = st["nch2"]
    NCH = nch1 + sum(nch2)
    np_p = [(offs[p + 1] - offs[p]) // 2 for p in range(npieces)]

    nc = bacc.Bacc(None, target_bir_lowering=False)

    xpair_d = nc.dram_tensor("xpair", [N_NODES // 2, 128], fp16,
                             kind="ExternalInput")
    xT_d = nc.dram_tensor("xT", [IN_F, NPC], f32r, kind="ExternalInput")
    w1c_d = nc.dram_tensor("w1c", [2 * IN_F, HID], f32r, kind="ExternalInput")
    w2c_d = nc.dram_tensor("w2c", [2 * HID, OUT_C], f32r, kind="ExternalInput")
    b1_d = nc.dram_tensor("b1c", [HID, 1], f32, kind="ExternalInput")
    b2_d = nc.dram_tensor("b2c", [OUT_C, 1], f32, kind="ExternalInput")
    iota_d = nc.dram_tensor("iota", [128, 128], fp16, kind="ExternalInput")
    ident_d = nc.dram_tensor("ident", [IN_F, IN_F], f32, kind="ExternalInput")
    invd_d = nc.dram_tensor("invd", [64, NPC], fp16, kind="ExternalInput")
    drel_d = nc.dram_tensor("dstrel", [128, NCH], f32, kind="ExternalInput")
    idx1_d = nc.dram_tensor("idx1", [128, st["S1"] // 16], i16,
                            kind="ExternalInput")
    idx2_d = nc.dram_tensor("idx2", [128, sum(st["S2"]) // 16], i16,
                            kind="ExternalInput")
    out_d = nc.dram_tensor("out", [OUT_C, NPC], fp16, kind="ExternalOutput")
    scratch_d = nc.dram_tensor("scratch", [1, 64], fp16)
    if debug:
        dbg_z1_d = nc.dram_tensor("dbg_z1", [128, NPC], f32,
                                  kind="ExternalOutput")
        dbg_z2_d = nc.dram_tensor("dbg_z2", [128, NPC], f32,
                                  kind="ExternalOutput")

    h_shard = [nc.dram_tensor(f"h_shard_{p}", [2 * np_p[p], HID], fp16)
               for p in range(npieces)]
    h_table = [nc.dram_tensor(f"h_table_{p}", [2 * 8 * np_p[p], HID], fp16,
                              addr_space="Shared")
               for p in range(npieces)]

    with tile.TileContext(nc) as tc:
        nc.gpsimd.load_library(library_config.mlp)
        with (
            tc.tile_pool(name="const", bufs=1) as cpool,
            tc.tile_pool(name="gath", bufs=int(os.environ.get("GBUFS", 3))) as gpool,
            tc.tile_pool(name="oh", bufs=int(os.environ.get("GBUFS", 3))) as ohpool,
            tc.tile_pool(name="hsb", bufs=12) as hpool,
            tc.tile_pool(name="osl", bufs=3) as opool,
            tc.tile_pool(name="agg", bufs=6, space="PSUM") as apool,
            tc.tile_pool(name="msc", bufs=1, space="PSUM") as mpool,
            tc.tile_pool(name="prj", bufs=1, space="PSUM") as jpool,
        ):
            # ---- persistent SBUF ----
            z1 = cpool.tile([2 * IN_F, NPC], f32r, tag="z1")
            z2 = cpool.tile([2 * HID, NPC], f32r, tag="z2")
            w1t = cpool.tile([2 * IN_F, HID], f32r, tag="w1t")
            w2t = cpool.tile([2 * HID, OUT_C], f32r, tag="w2t")
            b1t = cpool.tile([HID, 1], f32, tag="b1t")
            b2t = cpool.tile([OUT_C, 1], f32, tag="b2t")
            iot = cpool.tile([128, 128], fp16, tag="iot")
            idt = cpool.tile([IN_F, IN_F], f32, tag="idt")
            ivt = cpool.tile([128, NPC], fp16, tag="ivt")
            drt = cpool.tile([128, NCH], f32, tag="drt")
            ix1 = cpool.tile([128, st["S1"] // 16], i16, tag="ix1")
            ix2 = cpool.tile([128, sum(st["S2"]) // 16], i16, tag="ix2")

            # load order matters: the DMA engines are FIFO, so the first
            # gather call queues behind whatever consts precede it.  Load
            # only the L1-stream-critical prefix first (idx/dstrel split
            # so subtile deps release the first gather early); defer the
            # L2-only loads behind the first h write (SP queue blocks on
            # it, staggering them off the head of the DMA FIFO).
            # staged input loads: the DMA engines are FIFO, so anything
            # loaded before the gathers a piece-1 AllGather depends on
            # delays the whole collective chain.  stage0 = minimal stream
            # prefix; stage2 (after call 1, held back by a blocker DMA
            # reading call-1's onehot) = what layer-1 slabs 0..7 need;
            # stage3 (after piece-1's h is written) = everything else.
            IX1H = min(2560 // 16 * 16, st["S1"] // 16)
            DRTH = min(320, NCH)
            XTH = min(4096, NPC)
            nc.sync.dma_start(ix1[:, 0:IX1H], idx1_d[:, 0:IX1H])
            nc.sync.dma_start(iot[:], iota_d[:])
            nc.sync.dma_start(drt[:, 0:DRTH], drel_d[:, 0:DRTH])

            last_oh = [None]
            stage2 = [False]

            def load_stage2():
                nc.sync.dma_start(scratch_d[0:1, :],
                                  last_oh[0][0:1, 0, 0:64])
                nc.sync.dma_start(w1t[:], w1c_d[:])
                nc.sync.dma_start(b1t[:], b1_d[:])
                nc.sync.dma_start(z1[0:IN_F, 0:XTH], xT_d[:, 0:XTH])
                nc.sync.dma_start(ivt[64:128, 0:XTH], invd_d[:, 0:XTH])
                nc.sync.dma_start(idt[:], ident_d[:])
                stage2[0] = True

            deferred = [False]

            def load_deferred():
                nc.sync.dma_start(ix1[:, IX1H:], idx1_d[:, IX1H:])
                nc.sync.dma_start(drt[:, DRTH:], drel_d[:, DRTH:])
                nc.sync.dma_start(z1[0:IN_F, XTH:], xT_d[:, XTH:])
                nc.sync.dma_start(ivt[64:128, XTH:], invd_d[:, XTH:])
                nc.sync.dma_start(ix2[:], idx2_d[:])
                nc.sync.dma_start(w2t[:], w2c_d[:])
                nc.sync.dma_start(b2t[:], b2_d[:])
                deferred[0] = True

            # misc psum bank: [:, 0:64] = windows 96/97, [:, 64+64j] = L1
            # transpose slots
            misc = mpool.tile([128, SLAB], f32, tag="misc")

            # ---------------- shared machinery ----------------
            def win_slice(wtiles, w, wn):
                t = w // 2
                if t == 48:
                    tl = misc
                    col = 0
                else:
                    tl = wtiles[t // 8]
                    col = (t % 8) * 64
                ph = (w % 2) * 64
                return tl[ph: ph + 64, col: col + wn]

            def emit_stream(chunks_per_win, kinds, kbase, ixt, ix_off,
                            table_ap, wtiles, alloc_group, on_call_end,
                            depth=None, pipe=None):
                """Issue gather/onehot/matmul stream, software-pipelined:
                gathers + onehots run PIPE calls ahead of the matmuls, so
                a close-op stall at the DVE queue head never starves the
                PE.  Each window's psum accumulation group is contiguous
                WITHIN this stream.  Parity-pure chunks ('E'/'O') take one
                matmul with the matching lhsT half; mixed chunks use the
                folded dstrel + two."""
                chlist = []
                for w in range(NW):
                    for j in range(chunks_per_win[w]):
                        chlist.append((w, j))
                calls = []
                k = 0
                while k < len(chlist):
                    nb = min(GB // 128, len(chlist) - k)
                    calls.append((k, nb))
                    k += nb
                PIPE = pipe if pipe is not None \
                    else int(os.environ.get('PIPE', 1))
                DEP = depth if depth is not None \
                    else int(os.environ.get("GBUFS", 3))
                tiles = {}

                def front(ci):
                    k, nb = calls[ci]
                    b0 = k * 128
                    g = gpool.tile([128, GB // 128, 128], fp16,
                                   tag=f"g{DEP}", bufs=DEP, name="g")
                    nc.gpsimd.dma_gather(
                        out_ap=g[:, 0:nb, :],
                        in_ap=table_ap,
                        idxs_ap=ixt[:, ix_off + b0 // 16:
                                    ix_off + b0 // 16 + nb * 8],
                        num_idxs=nb * 128,
                        num_idxs_reg=nb * 128,
                        elem_size=128,
                        single_packet=False,
                    )
                    oh = ohpool.tile([128, GB // 128, 128], fp16,
                                     tag=f"oh{DEP}", bufs=DEP, name="oh")
                    last_oh[0] = oh
                    for col in range(nb):
                        kind = kinds[k + col]
                        ohw = 128 if kind == "M" else 64
                        # pure onehot (iota == dstrel), fp16, 4x DVE mode
                        nc.vector.tensor_scalar(
                            out=oh[:, col, 0:ohw],
                            in0=iot[:, 0:ohw],
                            scalar1=drt[:, kbase + k + col:
                                        kbase + k + col + 1],
                            scalar2=None,
                            op0=OP.is_equal,
                        )
                    tiles[ci] = (g, oh)

                def back(ci):
                    k, nb = calls[ci]
                    g, oh = tiles.pop(ci)
                    for col in range(nb):
                        w, j = chlist[k + col]
                        kind = kinds[k + col]
                        wn = min(WIN, NPC - w * WIN)
                        if (w // 2) != 48 and (w // 16) not in wtiles:
                            wtiles[w // 16] = alloc_group()
                        ps = win_slice(wtiles, w, wn)
                        first = (j == 0)
                        last = (j == chunks_per_win[w] - 1)
                        if kind == "M":
                            nc.tensor.matmul(
                                ps, g[:, col, 0:64], oh[:, col, 0:wn],
                                start=first, stop=False)
                            nc.tensor.matmul(
                                ps, g[:, col, 64:128],
                                oh[:, col, 64:64 + wn],
                                start=False, stop=last)
                        else:
                            half = slice(0, 64) if kind == "E" \
                                else slice(64, 128)
                            nc.tensor.matmul(
                                ps, g[:, col, half], oh[:, col, 0:wn],
                                start=first, stop=last)
                    on_call_end(ci + 1, chlist[k + nb - 1][0])

                for ci in range(len(calls)):
                    front(ci)
                    if ci >= PIPE:
                        back(ci - PIPE)
                for ci in range(max(0, len(calls) - PIPE), len(calls)):
                    back(ci)

            def close_groups(layer, z, wtiles, upto_g, state, add=False,
                             run_slabs=True):
                """Close whole 16-window psum groups <= upto_g: one strided
                ACT copy (or DVE add) per (bank tile, partition half)
                moves 8 windows at once; then run slab completions."""
                ng = 7  # groups 0..5 full tiles, group 6 = windows 96/97
                while state["g"] <= min(upto_g, ng - 1):
                    g = state["g"]
                    if g < 6:
                        tl = wtiles[g]
                        for ph in (0, 64):
                            # even (ph=0) / odd (ph=64) windows of group
                            zb = z[HID:, (16 * g + ph // 64) * WIN:
                                   (16 * g + ph // 64) * WIN + WIN]
                            zsl = AP(zb.tensor, zb.offset,
                                     [zb.ap[0], [128, 8], [1, WIN]])
                            pb = tl[ph: ph + 64, 0:SLAB]
                            psl = AP(pb.tensor, pb.offset,
                                     [pb.ap[0], [WIN, 8], [1, WIN]])
                            if not add:
                                nc.scalar.copy(zsl, psl)
                            else:
                                nc.vector.scalar_tensor_tensor(
                                    out=zsl, in0=psl, scalar=1.0, in1=zsl,
                                    op0=OP.mult, op1=OP.add)
                    else:
                        for w in (96, 97):
                            wn = min(WIN, NPC - w * WIN)
                            zsl = z[HID:, w * WIN: w * WIN + wn]
                            ps = win_slice(wtiles, w, wn)
                            if not add:
                                nc.scalar.copy(zsl, ps)
                            else:
                                nc.vector.scalar_tensor_tensor(
                                    out=zsl, in0=ps, scalar=1.0, in1=zsl,
                                    op0=OP.mult, op1=OP.add)
                    state["g"] += 1
                    if run_slabs:
                        for s in (2 * g, 2 * g + 1):
                            if s * SLAB < NPC:
                                finish_slab(layer, z, s)

            def finish_slab(layer, z, s):
                a, b = s * SLAB, min((s + 1) * SLAB, NPC)
                cols = b - a
                # mean division (invdeg folded out of the onehots)
                nc.vector.scalar_tensor_tensor(
                    out=z[HID:, a:b], in0=z[HID:, a:b], scalar=1.0,
                    in1=ivt[64:128, a:b],
                    op0=OP.mult, op1=OP.mult)
                if layer == 1:
                    p1 = jpool.tile([HID, SLAB], f32, tag="prj",
                                    name="prj")
                    nc.tensor.matmul(p1[:, :cols], w1t[:], z[:, a:b],
                                     start=True, stop=True)
                    nc.scalar.activation(z2[0:HID, a:b], p1[:, :cols],
                                         AF.Relu, bias=b1t[:, 0:1])
                    for j4 in range(-(-cols // 128)):
                        ca = a + j4 * 128
                        cb = min(ca + 128, b)
                        cc = cb - ca
                        tp = misc[:, 64 + (j4 % 4) * 64:
                                  128 + (j4 % 4) * 64]
                        nc.tensor.transpose(tp[:cc, :],
                                            z2[0:HID, ca:cb].bitcast(f32),
                                            idt[:])
                        hs = hpool.tile([128, HID], fp16, tag="hs",
                                        name="hs")
                        nc.scalar.copy(hs[:cc, :], tp[:cc, :])
                        # piece containing this chunk
                        p = next(i for i in range(npieces)
                                 if offs[i] <= ca < offs[i + 1])
                        nc.sync.dma_start(
                            h_shard[p][ca - offs[p]: cb - offs[p], :],
                            hs[:cc, :])
                    if not deferred[0] and b >= offs[1]:
                        load_deferred()
                    # queue AllGathers for completed h pieces (all but the
                    # last, which is deferred into the L2 piece-0 stream)
                    for p in range(npieces - 1):
                        if not ag_emitted[p] and b >= offs[p + 1]:
                            ag_pending.append(
                                (p, ag_ready[-1] if ag_ready else 0))
                            ag_emitted[p] = True
                else:
                    p2 = jpool.tile([HID, SLAB], f32, tag="prj",
                                    name="prj")[0:OUT_C, :]
                    nc.tensor.matmul(p2[:, :cols], w2t[:], z[:, a:b],
                                     start=True, stop=True)
                    osl = opool.tile([OUT_C, SLAB], fp16, tag="osl",
                                     name="osl")
                    nc.scalar.activation(osl[:, :cols], p2[:, :cols],
                                         AF.Identity, bias=b2t[:, 0:1])
                    nc.sync.dma_start(out_d[:, a:b], osl[:, :cols])

            def emit_ag(p):
                nc.gpsimd.collective_compute(
                    "AllGather",
                    mybir.AluOpType.bypass,
                    replica_groups=[list(range(M_CORES))],
                    ins=[h_shard[p][:]],
                    outs=[h_table[p][:]],
                )

            # ================= layer 1 =================
            wt1 = {}
            st1 = {"g": 0}
            ag_emitted = [False] * npieces
            ag_pending = []      # (piece, ready_at_call)
            ag_ready = []

            def alloc_agg():
                return apool.tile([128, SLAB], f32, tag="agg", name="agg")

            def l1_call_end(ncall, last_w):
                if ncall == 1 and not stage2[0]:
                    load_stage2()
                close_groups(1, z1, wt1, last_w // 16 - 1, st1)
                # emit pending AllGathers a few calls after their h piece
                # completed, so their sem waits never head-block the Pool
                # queue ahead of gather dispatches
                while ag_pending and ncall >= ag_pending[0][1] + AG_DELAY:
                    emit_ag(ag_pending.pop(0)[0])
                ag_ready.append(ncall)

            emit_stream(st["bud1"], st["kinds1"], 0, ix1, 0, xpair_d[:],
                        
            close_groups(1, z1, wt1, 6, st1)
            while ag_pending:
                emit_ag(ag_pending.pop(0)[0])

            # ================= layer 2 =================
            kbase = nch1
            ix_off = 0
            for p in range(npieces):
                wt2 = {}
                st2 = {"g": 0}
                lastp = (p == npieces - 1)

                def call_end(ncall, last_w, _p=p, _wt=wt2, _st=st2,
                             _lp=lastp):
                    if (_p == 0 and ncall == AG_AFTER_P1_CALLS
                            and not ag_emitted[npieces - 1]):
                        # last h piece is complete by now; emitting here
                        # keeps its sem wait from blocking the Pool SEQ
                        # ahead of the piece-0 gather dispatches
                        emit_ag(npieces - 1)
                        ag_emitted[npieces - 1] = True
                    close_groups(2, z2, _wt, last_w // 16 - 1, _st,
                                 add=(_p > 0), run_slabs=_lp)

                if p == npieces - 1 and not ag_emitted[p]:
                    emit_ag(p)          # safety: piece-0 stream was short
                    ag_emitted[p] = True
                base = h_table[p][:]
                tab = AP(base.tensor, 0, [[128, 8 * np_p[p]], [1, 128]])
                emit_stream(st["bud2"][p], st["kinds2"][p], kbase, ix2,
                            ix_off, tab, wt2, alloc_agg, call_end,
                            depth=6, pipe=1)
                close_groups(2, z2, wt2, 6, st2, add=(p > 0),
                             run_slabs=lastp)
                kbase += st["nch2"][p]
                ix_off += st["S2"][p] // 16
            if debug:
                nc.sync.dma_start(dbg_z1_d[:], z1[:].bitcast(f32))
                nc.sync.dma_start(dbg_z2_d[:], z2[:].bitcast(f32))

    nc.compile()
    return nc


def _make_in_maps(features, W_self1, W_neigh1, b1, W_self2, W_neigh2, b2,
                  st, pc):
    feat = np.ascontiguousarray(features, dtype=np.float32)
    xpair = feat.astype(np.float16).reshape(N_NODES // 2, 128)
    w1c = np.vstack([W_self1, W_neigh1]).astype(np.float32)
    w2c = np.vstack([W_self2, W_neigh2]).astype(np.float32)
    b1c = np.asarray(b1, np.float32).reshape(-1, 1)
    b2c = np.asarray(b2, np.float32).reshape(-1, 1)
    iota = np.tile(np.arange(128, dtype=np.float16), (128, 1))
    ident = np.eye(IN_F, dtype=np.float32)
    NW_ = NW
    in_maps = []
    for c in range(M_CORES):
        sl = slice(c * NPC, (c + 1) * NPC)
        ivd = np.ascontiguousarray(
            np.tile(pc["invdeg"][sl], (64, 1)).astype(np.float16))
        in_maps.append({
            "xpair": xpair,
            "xT": np.ascontiguousarray(feat[sl].T),
            "w1c": w1c, "w2c": w2c, "b1c": b1c, "b2c": b2c,
            "iota": iota, "ident": ident,
            "invd": ivd,
            "dstrel": pc["drt"][c],
            "idx1": pc["idx1"][c],
            "idx2": pc["idx2"][c],
        })
    return in_maps


_TRACE_RESULT = {}


def kernel(features, W_self1, W_neigh1, b1, W_self2, W_neigh2, b2, src, dst,
           _trace=False):
    from concourse.bass_utils import run_bass_kernel_spmd

    features = np.asarray(features, np.float32)
    src = np.asarray(src, np.int64)
    dst = np.asarray(dst, np.int64)

    # relabel nodes to flatten per-window in-degree sums (less padding)
    deg = np.bincount(dst, minlength=N_NODES)
    perm = _balance_permutation(deg)
    inv = np.empty(N_NODES, np.int64)
    inv[perm] = np.arange(N_NODES)

    st, pc = _prep(inv[src], inv[dst])
    nc = _build_bass(st)
    in_maps = _make_in_maps(features[perm], W_self1, W_neigh1, b1,
                            W_self2, W_neigh2, b2, st, pc)
    est_ns = None
    if _trace:
        try:
            from concourse.timeline_sim import TimelineSim
            ts = TimelineSim(nc, no_exec=True)
            ts.simulate()
            est_ns = int(ts.time)
        except Exception:
            import traceback
            traceback.print_exc()
    res = run_bass_kernel_spmd(nc, in_maps, core_ids=list(range(M_CORES)),
                               trace=False)
    exec_ns = res.exec_time_ns if res.exec_time_ns is not None else est_ns
    _TRACE_RESULT.clear()
    _TRACE_RESULT.update(dict(exec_time_ns=exec_ns,
                              trace=res.instructions_and_trace))
    out = np.concatenate([r["out"].T for r in res.results], axis=0)
    res_full = np.empty_like(out)
    res_full[perm] = out           # un-permute rows to original node ids
    return res_full.astype(np.float32)


# revision 12
# speedup vs baseline: 1.0515x; 1.0054x over previous
"""Trainium2 Bass kernel for a 2-layer mean-aggregation GraphSAGE GNN.

Strategy (8 NeuronCores, SPMD single program):
  - Destination nodes sharded contiguously across cores (6250/core), with
    a host-side node relabeling that (a) balances total in-degree across
    cores and (b) packs each 64-dst PSUM window to <= 1024 incoming edges
    (greedy + swap repair), so the max-over-cores window budgets hit the
    128-slot quantization exactly (L1 stream = 100096 slots vs 112512
    unbalanced).
  - Features / hidden activations stored as FP16 PAIR tables ([n/2, 128]
    rows = two consecutive node rows = 256B, the dma_gather granularity),
    so gather indices are pair ids < 32768 (int16) and no A/B table split
    is needed.  Parity (which pair half) is handled per chunk: where the
    even/odd split costs no padding the slots are parity-sorted and each
    chunk takes ONE fp16 matmul with the matching lhsT half; otherwise
    parity folds into the dstrel (+64) compared against a width-128 iota
    with two matmuls per chunk.
  - One-hots are built per chunk with tensor_scalar(is_equal) in fp16 —
    all-SBUF 2-byte operands hit the DVE 4x perf mode (~93ns/chunk).
    Mean division is applied per 512-col slab after the psum closes.
  - The gather/onehot/matmul stream is software-pipelined (front-end 2
    calls ahead), so close-op stalls at the DVE queue head never starve
    the PE; input loads are staged in three waves so the DMA-engine FIFO
    head stays clear for the gathers feeding the first AllGather.
  - PSUM: two windows per [128, 512] bank tile ((w%2)*64 partition half,
    ((w//2)%8)*64 column), rolling groups of 16 windows; each stream
    closes a group with ONE strided ACT copy (or DVE add for later
    layer-2 pieces) per partition half — accumulation groups stay
    contiguous per stream (required: split start/stop groups corrupt).
  - h exchange: 3 AllGathers over fp16 pair-row pieces of the local node
    range (boundaries chosen per input from candidates, slab-aligned),
    emitted a few calls after their h piece completes so their sem waits
    never head-block the Pool queue; layer-2 slot streams are split by
    src piece so gathers fire as each AllGather lands.
  - Projections via float32r matmuls over 512-col slabs (1 cycle/row).
  - Final [32, 6250] per-core output is transposed/concatenated and
    un-permuted on host.
"""

import os
import sys

import numpy as np

for _p in ("/opt/trn_rl_repo", "/root/.axon_site/_ro/trn_rl_repo"):
    if os.path.isdir(_p) and _p not in sys.path:
        sys.path.append(_p)

# ---- problem constants (hardcoded per harness contract) ----
N_NODES = 50000
N_EDGES = 800000
IN_F = 64
HID = 64
OUT_C = 32
M_CORES = 8
NPC = N_NODES // M_CORES     # 6250
WIN = 64
NW = -(-NPC // WIN)          # 98
GB = 2048                    # slots per dma_gather call
SLAB = 512                   # projection slab (psum bank cols)
# piece boundaries must be SLAB multiples (h DMAs never straddle pieces)
PIECE_CANDS = [(2048, 4096), (1536, 4096), (2048, 4608), (2560, 4608),
               (2048, 3584), (3072,), (3584,), (2560,)]
AG_AFTER_P1_CALLS = 8        # issue last AllGather after this many L2p1 calls
AG_DELAY = int(os.environ.get('AG_DELAY', 3))   # calls between h-piece completion and its AG
NS_PER_SLOT = 1.4225e-3      # µs, gather DMA cost per slot (cost model)


def _round_up(x, k):
    return (x + k - 1) // k * k


def _balance_permutation(deg):
    """Relabel nodes so per-core and per-window in-degree sums are flat,
    minimizing the round-128 padding of the max-over-cores window budgets.
    Returns perm (new position -> original node id)."""
    import heapq

    order = np.argsort(-deg, kind="stable")
    # 1) balance total in-degree across cores (capacity NPC each)
    heap = [(0, c) for c in range(M_CORES)]
    heapq.heapify(heap)
    cap = [NPC] * M_CORES
    core_nodes = [[] for _ in range(M_CORES)]
    for v in order:
        picked = []
        while True:
            s, c = heapq.heappop(heap)
            if cap[c] > 0:
                break
            picked.append((s, c))
        core_nodes[c].append(v)
        cap[c] -= 1
        heapq.heappush(heap, (s + int(deg[v]), c))
        for it in picked:
            heapq.heappush(heap, it)

    # 2) within each core, pack windows toward exact 1024-edge targets
    perm = np.empty(N_NODES, np.int64)
    lastcap = NPC - (NW - 1) * WIN
    for c in range(M_CORES):
        nodes = core_nodes[c]                     # desc by degree
        tot = int(deg[nodes].sum())
        t_last = max(tot - (NW - 1) * 8 * 128, 8 * int(deg[nodes[0]]))
        # heap of (-slack, w); assign heaviest node to max-slack window
        caps = [WIN] * (NW - 1) + [lastcap]
        tgts = [8 * 128] * (NW - 1) + [t_last]
        sums = [0] * NW
        cnts = [0] * NW
        heap2 = [(-tgts[w], w) for w in range(NW)]
        heapq.heapify(heap2)
        wassign = [[] for _ in range(NW)]
        for v in nodes:
            picked = []
            while True:
                negslack, w = heapq.heappop(heap2)
                if cnts[w] < caps[w]:
                    break
                picked.append((negslack, w))
            wassign[w].append(v)
            cnts[w] += 1
            sums[w] += int(deg[v])
            heapq.heappush(heap2, (sums[w] - tgts[w], w))
            for it in picked:
                heapq.heappush(heap2, it)
        # 3) repair pass: swap nodes between windows so no full window
        # exceeds the 1024-edge (8-chunk) budget bin; the short last
        # window may absorb up to its 768 bin.
        BIN = 8 * 128
        from collections import defaultdict
        wdeg = [defaultdict(list) for _ in range(NW)]   # deg -> [nodes]
        for w in range(NW):
            for v in wassign[w]:
                wdeg[w][int(deg[v])].append(v)

        def room(u):
            cap_u = BIN if u < NW - 1 else 768
            return cap_u - sums[u]

        for w in range(NW - 1):
            guard = 0
            while sums[w] > BIN and guard < 200:
                guard += 1
                need = sums[w] - BIN
                done = False
                for da in sorted(wdeg[w], reverse=True):
                    if not wdeg[w][da]:
                        continue
                    for k in range(min(need, da - 1), 0, -1):
                        db = da - k
                        for u in range(NW):
                            if (u == w or room(u) < k
                                    or not wdeg[u].get(db)):
                                continue
                            a = wdeg[w][da].pop()
                            b = wdeg[u][db].pop()
                            wdeg[w][db].append(b)
                            wdeg[u][da].append(a)
                            ia = wassign[w].index(a)
                            ib = wassign[u].index(b)
                            wassign[w][ia] = b
                            wassign[u][ib] = a
                            sums[w] -= k
                            sums[u] += k
                            done = True
                            break
                        if done:
                            break
                    if done:
                        break
                if not done:
                    break

        # 4) parity balance: within each window, order nodes so the
        # even-position half and odd-position half carry (<= bud/2) each
        # -> layer-1 chunks are parity-pure with zero extra padding
        for w in range(NW):
            nodes = sorted(wassign[w], key=lambda v: -deg[v])
            hcap = _round_up(max(sums[w], 1), 128) // 2
            bins = [[], []]
            bsum = [0, 0]
            bcap = [(len(nodes) + 1) // 2, len(nodes) // 2]
            for v in nodes:
                t = 0 if (bsum[0] <= bsum[1] and len(bins[0]) < bcap[0]) \
                    else (1 if len(bins[1]) < bcap[1] else 0)
                bins[t].append(v)
                bsum[t] += int(deg[v])
            for _ in range(64):
                hi = 0 if bsum[0] >= bsum[1] else 1
                excess = bsum[hi] - hcap
                if excess <= 0:
                    break
                done = False
                for a in sorted(bins[hi], key=lambda v: -deg[v]):
                    for b in sorted(bins[1 - hi], key=lambda v: deg[v]):
                        k = int(deg[a]) - int(deg[b])
                        if 1 <= k <= excess and bsum[1 - hi] + k <= hcap:
                            bins[hi].remove(a)
                            bins[1 - hi].remove(b)
                            bins[hi].append(b)
                            bins[1 - hi].append(a)
                            bsum[hi] -= k
                            bsum[1 - hi] += k
                            done = True
                            break
                    if done:
                        break
                if not done:
                    break
            merged = []
            for i in range(len(nodes)):
                merged.append(bins[i % 2][i // 2])
            wassign[w] = merged

        pos = c * NPC
        for w in range(NW):
            for v in wassign[w]:
                perm[pos] = v
                pos += 1
        assert pos == (c + 1) * NPC
    return perm


def _stream_layout(core, w, par, rel, gidx):
    """Parity-sorted padded slot streams for one gather stream.

    Within each window segment: even-parity slots first, padded to the
    shared boundary E_w = max-over-cores even count; odd slots after.
    Chunks below/above the boundary are parity-pure and take ONE matmul
    with the matching lhsT half; the (single) straddling chunk is mixed
    and uses the parity-folded dstrel + two matmuls.

    Returns (budgets_chunks[NW], kinds per chunk 'E'/'O'/'M',
             idx [M, S], drl [M, S], S).
    """
    ce = np.zeros((M_CORES, NW), np.int64)
    co = np.zeros((M_CORES, NW), np.int64)
    np.add.at(ce, (core[par == 0], w[par == 0]), 1)
    np.add.at(co, (core[par == 1], w[par == 1]), 1)
    E = ce.max(axis=0)
    O = co.max(axis=0)
    J = (ce + co).max(axis=0)
    bud = np.maximum(_round_up(J, 128), 128)
    # sorted mode only where the parity split costs no extra padding;
    # otherwise fold parity into dstrel (all chunks mixed)
    smode = _round_up(E + O, 128) <= bud
    # even/odd boundary: chunk-aligned when the odds still fit
    B = np.where(_round_up(E, 128) + O <= bud, _round_up(E, 128), E)
    B = np.where(smode, B, 0)
    seg_off = np.concatenate([[0], np.cumsum(bud)])
    S = int(seg_off[-1])

    # chunk kinds (shared across cores)
    kinds = []
    for wi in range(NW):
        nch = int(bud[wi]) // 128
        for j in range(nch):
            lo, hi = j * 128, (j + 1) * 128
            if not smode[wi]:
                kinds.append("M")
            elif hi <= B[wi]:
                kinds.append("E")
            elif lo >= B[wi]:
                kinds.append("O")
            else:
                kinds.append("M")

    # slot positions: sorted windows rank evens from seg start, odds
    # from B; folded windows rank jointly
    pcls = np.where(smode[w], par, 0)
    key = (core * NW + w) * 2 + pcls
    order = np.argsort(key, kind="stable")
    ks = key[order]
    grp_start = np.searchsorted(ks, np.arange(M_CORES * NW * 2), side="left")
    ranks = np.arange(len(ks)) - grp_start[ks]
    wo = w[order]
    po = par[order]
    pos = seg_off[wo] + np.where(smode[wo] & (po == 1), B[wo], 0) + ranks

    # dstrel: fold +64*parity for folded windows and mixed-chunk odds
    ch_in_seg = (pos - seg_off[wo]) // 128
    mixed = (ch_in_seg * 128 < B[wo]) & ((ch_in_seg + 1) * 128 > B[wo])
    fold = (~smode[wo]) | (mixed & (po == 1))
    drl_o = np.where(fold & (po == 1), rel[order] + WIN,
                     rel[order]).astype(np.float32)

    idx_buf = np.zeros((M_CORES, S), np.int64)
    drl_buf = np.full((M_CORES, S), -1.0, np.float32)
    idx_buf[core[order], pos] = gidx[order]
    drl_buf[core[order], pos] = drl_o
    return [int(b) // 128 for b in bud], kinds, idx_buf, drl_buf, S


def _wrap_idx(streams):
    """[M, S] int -> per-core [128, S/16] int16 gather-index layout."""
    res = []
    for c in range(M_CORES):
        a = streams[c].astype(np.int16).reshape(-1, 16).T
        res.append(np.ascontiguousarray(np.tile(a, (8, 1))))
    return res


def _prep(src, dst):
    deg = np.bincount(dst, minlength=N_NODES).astype(np.int64)
    invdeg = (1.0 / np.maximum(deg, 1.0)).astype(np.float32)

    core = dst // NPC
    dloc = dst % NPC
    w_e = dloc // WIN
    par = (src & 1).astype(np.int64)
    drel = (dloc % WIN + WIN * par).astype(np.float32)
    sloc = src % NPC
    scor = src // NPC

    # ---- L1: single stream, gather from global x pair table ----
    rel = (dloc % WIN).astype(np.float32)
    bud1, kinds1, idx1_buf, drl1_buf, S1 = _stream_layout(
        core, w_e, par, rel, src >> 1)

    # ---- choose L2 piece boundaries (min estimated critical chain) ----
    l1_dma = S1 * NS_PER_SLOT            # µs
    best = None
    for bounds in PIECE_CANDS:
        offs = [0] + list(bounds) + [NPC]
        piece_slots = []
        tot = 0
        for p in range(len(offs) - 1):
            sel = (sloc >= offs[p]) & (sloc < offs[p + 1])
            c = np.zeros((M_CORES, NW), np.int64)
            np.add.at(c, (core[sel], w_e[sel]), 1)
            b = _round_up(c.max(axis=0), 128)
            if p == 0:
                b = np.maximum(b, 128)
            piece_slots.append(int(b.sum()))
            tot += int(b.sum())
        # chain: AGs serialize on the collective resource; last piece's
        # gathers wait for its AG
        t = 0.0
        for p in range(len(offs) - 1):
            ready = offs[p + 1] / NPC * l1_dma + 18.0
            nodes = offs[p + 1] - offs[p]
            ag = 15.0 + nodes * 8 * 128 / 40e3
            t = max(t, ready) + ag
        span = max(t + piece_slots[-1] * NS_PER_SLOT + 12.0,
                   (S1 + tot) * NS_PER_SLOT + 25.0)
        if best is None or span < best[0]:
            best = (span, offs)
    if os.environ.get("FORCE_OFFS"):
        best = (0, [0] + [int(x) for x in
                          os.environ["FORCE_OFFS"].split(",")] + [NPC])
    offs = best[1]
    npieces = len(offs) - 1
    assert all(o % SLAB == 0 for o in offs[1:-1])

    # ---- L2: one stream per piece, gather from h pair-piece tables ----
    bud2, kinds2, S2 = [], [], []
    idx2_bufs, drl2_bufs = [], []
    for p in range(npieces):
        np_p = (offs[p + 1] - offs[p]) // 2          # pairs/core in piece
        sel = (sloc >= offs[p]) & (sloc < offs[p + 1])
        gidx = scor[sel] * np_p + (sloc[sel] - offs[p]) // 2
        assert gidx.max() < 8 * np_p <= 32768
        b, kn, ib, db, S = _stream_layout(
            core[sel], w_e[sel], par[sel], rel[sel], gidx)
        bud2.append(b)
        kinds2.append(kn)
        idx2_bufs.append(ib)
        drl2_bufs.append(db)
        S2.append(S)

    st = dict(
        offs=offs, npieces=npieces,
        bud1=bud1, bud2=bud2,
        kinds1=kinds1, kinds2=kinds2,
        S1=S1, S2=S2,
        nch1=S1 // 128, nch2=[s // 128 for s in S2],
    )

    # per-core drt: [128, nch1 + sum(nch2)] f32 (L1 cols then L2 pieces)
    drt = []
    for c in range(M_CORES):
        cols = [drl1_buf[c].reshape(-1, 128).T]
        for p in range(npieces):
            cols.append(drl2_bufs[p][c].reshape(-1, 128).T)
        drt.append(np.ascontiguousarray(np.concatenate(cols, axis=1)))

    idx1 = _wrap_idx(idx1_buf)
    idx2 = []
    for c in range(M_CORES):
        blocks = [idx2_bufs[p][c] for p in range(npieces)]
        flat = np.concatenate(blocks)
        a = flat.astype(np.int16).reshape(-1, 16).T
        idx2.append(np.ascontiguousarray(np.tile(a, (8, 1))))

    pc = dict(drt=drt, idx1=idx1, idx2=idx2, invdeg=invdeg)
    return st, pc


def _build_bass(st, debug=False):
    import concourse.bass as bass
    import concourse.mybir as mybir
    import concourse.tile as tile
    from concourse.ap import AP
    from concourse import bacc, library_config

    f32 = mybir.dt.float32
    f32r = mybir.dt.float32r
    fp16 = mybir.dt.float16
    i16 = mybir.dt.int16
    AF = mybir.ActivationFunctionType
    OP = mybir.AluOpType

    offs = st["offs"]
    npieces = st["npieces"]
    nch1 = st["nch1"]
    nch2 = st["nch2"]
    NCH = nch1 + sum(nch2)
    np_p = [(offs[p + 1] - offs[p]) // 2 for p in range(npieces)]

    nc = bacc.Bacc(None, target_bir_lowering=False)

    xpair_d = nc.dram_tensor("xpair", [N_NODES // 2, 128], fp16,
                             kind="ExternalInput")
    xT_d = nc.dram_tensor("xT", [IN_F, NPC], fp16, kind="ExternalInput")
    w1c_d = nc.dram_tensor("w1c", [2 * IN_F, HID], f32r, kind="ExternalInput")
    w2c_d = nc.dram_tensor("w2c", [2 * HID, OUT_C], f32r, kind="ExternalInput")
    b1_d = nc.dram_tensor("b1c", [HID, 1], f32, kind="ExternalInput")
    b2_d = nc.dram_tensor("b2c", [OUT_C, 1], f32, kind="ExternalInput")
    iota_d = nc.dram_tensor("iota", [128, 128], fp16, kind="ExternalInput")
    ident_d = nc.dram_tensor("ident", [IN_F, IN_F], f32, kind="ExternalInput")
    invd_d = nc.dram_tensor("invd", [64, NPC], fp16, kind="ExternalInput")
    drel_d = nc.dram_tensor("dstrel", [128, NCH], f32, kind="ExternalInput")
    idx1_d = nc.dram_tensor("idx1", [128, st["S1"] // 16], i16,
                            kind="ExternalInput")
    idx2_d = nc.dram_tensor("idx2", [128, sum(st["S2"]) // 16], i16,
                            kind="ExternalInput")
    out_d = nc.dram_tensor("out", [OUT_C, NPC], fp16, kind="ExternalOutput")
    scratch_d = nc.dram_tensor("scratch", [1, 64], fp16)
    if debug:
        dbg_z1_d = nc.dram_tensor("dbg_z1", [128, NPC], f32,
                                  kind="ExternalOutput")
        dbg_z2_d = nc.dram_tensor("dbg_z2", [128, NPC], f32,
                                  kind="ExternalOutput")

    h_shard = [nc.dram_tensor(f"h_shard_{p}", [2 * np_p[p], HID], fp16)
               for p in range(npieces)]
    h_table = [nc.dram_tensor(f"h_table_{p}", [2 * 8 * np_p[p], HID], fp16,
                              addr_space="Shared")
               for p in range(npieces)]

    with tile.TileContext(nc) as tc:
        nc.gpsimd.load_library(library_config.mlp)
        with (
            tc.tile_pool(name="const", bufs=1) as cpool,
            tc.tile_pool(name="gath", bufs=int(os.environ.get("GBUFS", 3))) as gpool,
            tc.tile_pool(name="oh", bufs=int(os.environ.get("GBUFS", 3))) as ohpool,
            tc.tile_pool(name="hsb", bufs=12) as hpool,
            tc.tile_pool(name="osl", bufs=3) as opool,
            tc.tile_pool(name="agg", bufs=6, space="PSUM") as apool,
            tc.tile_pool(name="msc", bufs=1, space="PSUM") as mpool,
            tc.tile_pool(name="prj", bufs=1, space="PSUM") as jpool,
        ):
            # ---- persistent SBUF ----
            z1 = cpool.tile([2 * IN_F, NPC], f32r, tag="z1")
            z2 = cpool.tile([2 * HID, NPC], f32r, tag="z2")
            w1t = cpool.tile([2 * IN_F, HID], f32r, tag="w1t")
            w2t = cpool.tile([2 * HID, OUT_C], f32r, tag="w2t")
            b1t = cpool.tile([HID, 1], f32, tag="b1t")
            b2t = cpool.tile([OUT_C, 1], f32, tag="b2t")
            iot = cpool.tile([128, 128], fp16, tag="iot")
            idt = cpool.tile([IN_F, IN_F], f32, tag="idt")
            ivt = cpool.tile([128, NPC], fp16, tag="ivt")
            drt = cpool.tile([128, NCH], f32, tag="drt")
            ix1 = cpool.tile([128, st["S1"] // 16], i16, tag="ix1")
            ix2 = cpool.tile([128, sum(st["S2"]) // 16], i16, tag="ix2")
            xts = cpool.tile([IN_F, NPC], fp16, tag="xts")

            # load order matters: the DMA engines are FIFO, so the first
            # gather call queues behind whatever consts precede it.  Load
            # only the L1-stream-critical prefix first (idx/dstrel split
            # so subtile deps release the first gather early); defer the
            # L2-only loads behind the first h write (SP queue blocks on
            # it, staggering them off the head of the DMA FIFO).
            # staged input loads: the DMA engines are FIFO, so anything
            # loaded before the gathers a piece-1 AllGather depends on
            # delays the whole collective chain.  stage0 = minimal stream
            # prefix; stage2 (after call 1, held back by a blocker DMA
            # reading call-1's onehot) = what layer-1 slabs 0..7 need;
            # stage3 (after piece-1's h is written) = everything else.
            IX1H = min(2560 // 16 * 16, st["S1"] // 16)
            DRTH = min(320, NCH)
            XTH = min(4096, NPC)
            nc.sync.dma_start(ix1[:, 0:IX1H], idx1_d[:, 0:IX1H])
            nc.sync.dma_start(iot[:], iota_d[:])
            nc.sync.dma_start(drt[:, 0:DRTH], drel_d[:, 0:DRTH])

            last_oh = [None]
            stage2 = [False]

            def load_stage2():
                nc.sync.dma_start(scratch_d[0:1, :],
                                  last_oh[0][0:1, 0, 0:64])
                nc.sync.dma_start(w1t[:], w1c_d[:])
                nc.sync.dma_start(b1t[:], b1_d[:])
                nc.sync.dma_start(xts[:, 0:XTH], xT_d[:, 0:XTH])
                nc.scalar.copy(z1[0:IN_F, 0:XTH], xts[0:IN_F, 0:XTH])
                nc.sync.dma_start(ivt[64:128, 0:XTH], invd_d[:, 0:XTH])
                nc.sync.dma_start(idt[:], ident_d[:])
                stage2[0] = True

            deferred = [False]

            def load_deferred():
                nc.sync.dma_start(ix1[:, IX1H:], idx1_d[:, IX1H:])
                nc.sync.dma_start(drt[:, DRTH:], drel_d[:, DRTH:])
                nc.sync.dma_start(xts[:, XTH:], xT_d[:, XTH:])
                nc.scalar.copy(z1[0:IN_F, XTH:], xts[0:IN_F, XTH:])
                nc.sync.dma_start(ivt[64:128, XTH:], invd_d[:, XTH:])
                nc.sync.dma_start(ix2[:], idx2_d[:])
                nc.sync.dma_start(w2t[:], w2c_d[:])
                nc.sync.dma_start(b2t[:], b2_d[:])
                deferred[0] = True

            # misc psum bank: [:, 0:64] = windows 96/97, [:, 64+64j] = L1
            # transpose slots
            misc = mpool.tile([128, SLAB], f32, tag="misc")

            # ---------------- shared machinery ----------------
            def win_slice(wtiles, w, wn):
                t = w // 2
                if t == 48:
                    tl = misc
                    col = 0
                else:
                    tl = wtiles[t // 8]
                    col = (t % 8) * 64
                ph = (w % 2) * 64
                return tl[ph: ph + 64, col: col + wn]

            def emit_stream(chunks_per_win, kinds, kbase, ixt, ix_off,
                            table_ap, wtiles, alloc_group, on_call_end,
                            depth=None, pipe=None):
                """Issue gather/onehot/matmul stream, software-pipelined:
                gathers + onehots run PIPE calls ahead of the matmuls, so
                a close-op stall at the DVE queue head never starves the
                PE.  Each window's psum accumulation group is contiguous
                WITHIN this stream.  Parity-pure chunks ('E'/'O') take one
                matmul with the matching lhsT half; mixed chunks use the
                folded dstrel + two."""
                chlist = []
                for w in range(NW):
                    for j in range(chunks_per_win[w]):
                        chlist.append((w, j))
                calls = []
                k = 0
                while k < len(chlist):
                    nb = min(GB // 128, len(chlist) - k)
                    calls.append((k, nb))
                    k += nb
                PIPE = pipe if pipe is not None \
                    else int(os.environ.get('PIPE', 1))
                DEP = depth if depth is not None \
                    else int(os.environ.get("GBUFS", 3))
                tiles = {}

                def front(ci):
                    k, nb = calls[ci]
                    b0 = k * 128
                    g = gpool.tile([128, GB // 128, 128], fp16,
                                   tag=f"g{DEP}", bufs=DEP, name="g")
                    nc.gpsimd.dma_gather(
                        out_ap=g[:, 0:nb, :],
                        in_ap=table_ap,
                        idxs_ap=ixt[:, ix_off + b0 // 16:
                                    ix_off + b0 // 16 + nb * 8],
                        num_idxs=nb * 128,
                        num_idxs_reg=nb * 128,
                        elem_size=128,
                        single_packet=False,
                    )
                    oh = ohpool.tile([128, GB // 128, 128], fp16,
                                     tag=f"oh{DEP}", bufs=DEP, name="oh")
                    last_oh[0] = oh
                    for col in range(nb):
                        kind = kinds[k + col]
                        ohw = 128 if kind == "M" else 64
                        # pure onehot (iota == dstrel), fp16, 4x DVE mode
                        nc.vector.tensor_scalar(
                            out=oh[:, col, 0:ohw],
                            in0=iot[:, 0:ohw],
                            scalar1=drt[:, kbase + k + col:
                                        kbase + k + col + 1],
                            scalar2=None,
                            op0=OP.is_equal,
                        )
                    tiles[ci] = (g, oh)

                def back(ci):
                    k, nb = calls[ci]
                    g, oh = tiles.pop(ci)
                    for col in range(nb):
                        w, j = chlist[k + col]
                        kind = kinds[k + col]
                        wn = min(WIN, NPC - w * WIN)
                        if (w // 2) != 48 and (w // 16) not in wtiles:
                            wtiles[w // 16] = alloc_group()
                        ps = win_slice(wtiles, w, wn)
                        first = (j == 0)
                        last = (j == chunks_per_win[w] - 1)
                        if kind == "M":
                            nc.tensor.matmul(
                                ps, g[:, col, 0:64], oh[:, col, 0:wn],
                                start=first, stop=False)
                            nc.tensor.matmul(
                                ps, g[:, col, 64:128],
                                oh[:, col, 64:64 + wn],
                                start=False, stop=last)
                        else:
                            half = slice(0, 64) if kind == "E" \
                                else slice(64, 128)
                            nc.tensor.matmul(
                                ps, g[:, col, half], oh[:, col, 0:wn],
                                start=first, stop=last)
                    on_call_end(ci + 1, chlist[k + nb - 1][0])

                for ci in range(len(calls)):
                    front(ci)
                    if ci >= PIPE:
                        back(ci - PIPE)
                for ci in range(max(0, len(calls) - PIPE), len(calls)):
                    back(ci)

            def close_groups(layer, z, wtiles, upto_g, state, add=False,
                             run_slabs=True):
                """Close whole 16-window psum groups <= upto_g: one strided
                ACT copy (or DVE add) per (bank tile, partition half)
                moves 8 windows at once; then run slab completions."""
                ng = 7  # groups 0..5 full tiles, group 6 = windows 96/97
                while state["g"] <= min(upto_g, ng - 1):
                    g = state["g"]
                    if g < 6:
                        tl = wtiles[g]
                        for ph in (0, 64):
                            # even (ph=0) / odd (ph=64) windows of group
                            zb = z[HID:, (16 * g + ph // 64) * WIN:
                                   (16 * g + ph // 64) * WIN + WIN]
                            zsl = AP(zb.tensor, zb.offset,
                                     [zb.ap[0], [128, 8], [1, WIN]])
                            pb = tl[ph: ph + 64, 0:SLAB]
                            psl = AP(pb.tensor, pb.offset,
                                     [pb.ap[0], [WIN, 8], [1, WIN]])
                            if not add:
                                nc.scalar.copy(zsl, psl)
                            else:
                                nc.vector.scalar_tensor_tensor(
                                    out=zsl, in0=psl, scalar=1.0, in1=zsl,
                                    op0=OP.mult, op1=OP.add)
                    else:
                        for w in (96, 97):
                            wn = min(WIN, NPC - w * WIN)
                            zsl = z[HID:, w * WIN: w * WIN + wn]
                            ps = win_slice(wtiles, w, wn)
                            if not add:
                                nc.scalar.copy(zsl, ps)
                            else:
                                nc.vector.scalar_tensor_tensor(
                                    out=zsl, in0=ps, scalar=1.0, in1=zsl,
                                    op0=OP.mult, op1=OP.add)
                    state["g"] += 1
                    if run_slabs:
                        for s in (2 * g, 2 * g + 1):
                            if s * SLAB < NPC:
                                finish_slab(layer, z, s)

            def finish_slab(layer, z, s):
                a, b = s * SLAB, min((s + 1) * SLAB, NPC)
                cols = b - a
                # mean division (invdeg folded out of the onehots)
                nc.vector.scalar_tensor_tensor(
                    out=z[HID:, a:b], in0=z[HID:, a:b], scalar=1.0,
                    in1=ivt[64:128, a:b],
                    op0=OP.mult, op1=OP.mult)
                if layer == 1:
                    p1 = jpool.tile([HID, SLAB], f32, tag="prj",
                                    name="prj")
                    nc.tensor.matmul(p1[:, :cols], w1t[:], z[:, a:b],
                                     start=True, stop=True)
                    nc.scalar.activation(z2[0:HID, a:b], p1[:, :cols],
                                         AF.Relu, bias=b1t[:, 0:1])
                    for j4 in range(-(-cols // 128)):
                        ca = a + j4 * 128
                        cb = min(ca + 128, b)
                        cc = cb - ca
                        tp = misc[:, 64 + (j4 % 4) * 64:
                                  128 + (j4 % 4) * 64]
                        nc.tensor.transpose(tp[:cc, :],
                                            z2[0:HID, ca:cb].bitcast(f32),
                                            idt[:])
                        hs = hpool.tile([128, HID], fp16, tag="hs",
                                        name="hs")
                        nc.scalar.copy(hs[:cc, :], tp[:cc, :])
                        # piece containing this chunk
                        p = next(i for i in range(npieces)
                                 if offs[i] <= ca < offs[i + 1])
                        nc.sync.dma_start(
                            h_shard[p][ca - offs[p]: cb - offs[p], :],
                            hs[:cc, :])
                    if not deferred[0] and b >= offs[1]:
                        load_deferred()
                    # queue AllGathers for completed h pieces (all but the
                    # last, which is deferred into the L2 piece-0 stream)
                    for p in range(npieces - 1):
                        if not ag_emitted[p] and b >= offs[p + 1]:
                            ag_pending.append(
                                (p, ag_ready[-1] if ag_ready else 0))
                            ag_emitted[p] = True
                else:
                    p2 = jpool.tile([HID, SLAB], f32, tag="prj",
                                    name="prj")[0:OUT_C, :]
                    nc.tensor.matmul(p2[:, :cols], w2t[:], z[:, a:b],
                                     start=True, stop=True)
                    osl = opool.tile([OUT_C, SLAB], fp16, tag="osl",
                                     name="osl")
                    nc.scalar.activation(osl[:, :cols], p2[:, :cols],
                                         AF.Identity, bias=b2t[:, 0:1])
                    nc.sync.dma_start(out_d[:, a:b], osl[:, :cols])

            def emit_ag(p):
                nc.gpsimd.collective_compute(
                    "AllGather",
                    mybir.AluOpType.bypass,
                    replica_groups=[list(range(M_CORES))],
                    ins=[h_shard[p][:]],
                    outs=[h_table[p][:]],
                )

            # ================= layer 1 =================
            wt1 = {}
            st1 = {"g": 0}
            ag_emitted = [False] * npieces
            ag_pending = []      # (piece, ready_at_call)
            ag_ready = []

            def alloc_agg():
                return apool.tile([128, SLAB], f32, tag="agg", name="agg")

            def l1_call_end(ncall, last_w):
                if ncall == 1 and not stage2[0]:
                    load_stage2()
                close_groups(1, z1, wt1, last_w // 16 - 1, st1)
                # emit pending AllGathers a few calls after their h piece
                # completed, so their sem waits never head-block the Pool
                # queue ahead of gather dispatches
                while ag_pending and ncall >= ag_pending[0][1] + AG_DELAY:
                    emit_ag(ag_pending.pop(0)[0])
                ag_ready.append(ncall)

            emit_stream(st["bud1"], st["kinds1"], 0, ix1, 0, xpair_d[:],
                        
            close_groups(1, z1, wt1, 6, st1)
            while ag_pending:
                emit_ag(ag_pending.pop(0)[0])

            # ================= layer 2 =================
            kbase = nch1
            ix_off = 0
            for p in range(npieces):
                wt2 = {}
                st2 = {"g": 0}
                lastp = (p == npieces - 1)

                def call_end(ncall, last_w, _p=p, _wt=wt2, _st=st2,
                             _lp=lastp):
                    if (_p == 0 and ncall == AG_AFTER_P1_CALLS
                            and not ag_emitted[npieces - 1]):
                        # last h piece is complete by now; emitting here
                        # keeps its sem wait from blocking the Pool SEQ
                        # ahead of the piece-0 gather dispatches
                        emit_ag(npieces - 1)
                        ag_emitted[npieces - 1] = True
                    close_groups(2, z2, _wt, last_w // 16 - 1, _st,
                                 add=(_p > 0), run_slabs=_lp)

                if p == npieces - 1 and not ag_emitted[p]:
                    emit_ag(p)          # safety: piece-0 stream was short
                    ag_emitted[p] = True
                base = h_table[p][:]
                tab = AP(base.tensor, 0, [[128, 8 * np_p[p]], [1, 128]])
                emit_stream(st["bud2"][p], st["kinds2"][p], kbase, ix2,
                            ix_off, tab, wt2, alloc_agg, call_end,
                            depth=6, pipe=1)
                close_groups(2, z2, wt2, 6, st2, add=(p > 0),
                             run_slabs=lastp)
                kbase += st["nch2"][p]
                ix_off += st["S2"][p] // 16
            if debug:
                nc.sync.dma_start(dbg_z1_d[:], z1[:].bitcast(f32))
                nc.sync.dma_start(dbg_z2_d[:], z2[:].bitcast(f32))

    nc.compile()
    return nc


def _make_in_maps(features, W_self1, W_neigh1, b1, W_self2, W_neigh2, b2,
                  st, pc):
    feat = np.ascontiguousarray(features, dtype=np.float32)
    xpair = feat.astype(np.float16).reshape(N_NODES // 2, 128)
    w1c = np.vstack([W_self1, W_neigh1]).astype(np.float32)
    w2c = np.vstack([W_self2, W_neigh2]).astype(np.float32)
    b1c = np.asarray(b1, np.float32).reshape(-1, 1)
    b2c = np.asarray(b2, np.float32).reshape(-1, 1)
    iota = np.tile(np.arange(128, dtype=np.float16), (128, 1))
    ident = np.eye(IN_F, dtype=np.float32)
    NW_ = NW
    in_maps = []
    for c in range(M_CORES):
        sl = slice(c * NPC, (c + 1) * NPC)
        ivd = np.ascontiguousarray(
            np.tile(pc["invdeg"][sl], (64, 1)).astype(np.float16))
        in_maps.append({
            "xpair": xpair,
            "xT": np.ascontiguousarray(feat[sl].T.astype(np.float16)),
            "w1c": w1c, "w2c": w2c, "b1c": b1c, "b2c": b2c,
            "iota": iota, "ident": ident,
            "invd": ivd,
            "dstrel": pc["drt"][c],
            "idx1": pc["idx1"][c],
            "idx2": pc["idx2"][c],
        })
    return in_maps


_TRACE_RESULT = {}


def kernel(features, W_self1, W_neigh1, b1, W_self2, W_neigh2, b2, src, dst,
           _trace=False):
    from concourse.bass_utils import run_bass_kernel_spmd

    features = np.asarray(features, np.float32)
    src = np.asarray(src, np.int64)
    dst = np.asarray(dst, np.int64)

    # relabel nodes to flatten per-window in-degree sums (less padding)
    deg = np.bincount(dst, minlength=N_NODES)
    perm = _balance_permutation(deg)
    inv = np.empty(N_NODES, np.int64)
    inv[perm] = np.arange(N_NODES)

    st, pc = _prep(inv[src], inv[dst])
    nc = _build_bass(st)
    in_maps = _make_in_maps(features[perm], W_self1, W_neigh1, b1,
                            W_self2, W_neigh2, b2, st, pc)
    est_ns = None
    if _trace:
        try:
            from concourse.timeline_sim import TimelineSim
            ts = TimelineSim(nc, no_exec=True)
            ts.simulate()
            est_ns = int(ts.time)
        except Exception:
            import traceback
            traceback.print_exc()
    res = run_bass_kernel_spmd(nc, in_maps, core_ids=list(range(M_CORES)),
                               trace=False)
    exec_ns = res.exec_time_ns if res.exec_time_ns is not None else est_ns
    _TRACE_RESULT.clear()
    _TRACE_RESULT.update(dict(exec_time_ns=exec_ns,
                              trace=res.instructions_and_trace))
    out = np.concatenate([r["out"].T for r in res.results], axis=0)
    res_full = np.empty_like(out)
    res_full[perm] = out           # un-permute rows to original node ids
    return res_full.astype(np.float32)
